# revision 11
# baseline (speedup 1.0000x reference)
"""BWGNN_Hetero Trainium2 kernel (8 NeuronCores, SPMD).

Math restructure of the reference:
  - poly_conv with THETAS (Bernstein, D=2) shares the Krylov basis
    P0 = h, P1 = L~ P0, P2 = L~ P1 across all three thetas, where
    L~ x = x - dinv * segsum((x*dinv)[src], dst).
  - concat(h_theta) @ W3 collapses to sum_k P_k @ V_k with
    V_k = sum_i THETA[i][k] * W3[i*H:(i+1)*H] (host-precomputed).
  - out = leaky(sum_r sum_k P_k^r @ V_k + 2*b3)

Sharding: nodes split 8 ways (6250/core, padded to 6272 = 49*128).
Edges live on their dst-owner core, grouped per (64-node dst block,
src-parity) cell. The scaled-feature table is pair-packed: table row
j = [node 2j | node 2j+1] (256B rows, 25088 rows < int16 range), so
gathers use stride 256B with base offset parity*128B and the
all-gather moves only the compact payload (0.8MB/core per hop).
Gathers are issued as large calls (14 cells = 4480 rows each) to
amortize the ~1us SWDGE fixed cost; aggregation is one-hot matmuls
accumulating 7 blocks into a single PSUM bank, with chunked [64,448]
finish ops.
"""

import math
import os
import sys

import ml_dtypes
import numpy as np

for _p in ("/opt/trn_rl_repo", "/root/.axon_site/_ro/trn_rl_repo"):
    if os.path.isdir(_p) and _p not in sys.path:
        sys.path.insert(0, _p)

import concourse.bacc as bacc
import concourse.bass as bass
import concourse.mybir as mybir
import concourse.tile as tile
from concourse.bass_utils import run_bass_kernel_spmd

F32 = mybir.dt.float32
BF16 = mybir.dt.bfloat16
I16 = mybir.dt.int16
AF = mybir.ActivationFunctionType


class Cfg:
    def __init__(self, N=50000, F=128, H=64, NCORES=8):
        self.N = N
        self.F = F
        self.H = H
        self.NCORES = NCORES
        self.NPC = N // NCORES              # real nodes per core
        self.BLK = 64                       # dst nodes per aggregation block
        self.NBLK = -(-self.NPC // self.BLK)
        self.NPAD = self.NBLK * self.BLK    # padded nodes per core
        assert self.NPAD % 128 == 0
        self.NTAB = self.NPAD * NCORES      # padded global table rows
        self.NROW = self.NTAB // 2          # pair-packed table rows
        assert self.NROW <= 32768, "int16 gather index range"
        self.G = 7                          # dst blocks per gather call
        assert self.NBLK % self.G == 0
        self.NGRP = self.NBLK // self.G     # call groups per (relhop, parity)
        self.GW = self.G * self.BLK         # finish-chunk width (448)


THETAS = None


def _calc_thetas(d=2):
    thetas = []
    for i in range(d + 1):
        p1 = np.zeros(i + 1)
        p1[i] = 0.5 ** i
        m = d - i
        p2 = np.array([math.comb(m, j) * (-0.5) ** j for j in range(m + 1)])
        c = np.convolve(p1, p2)
        beta = math.factorial(i) * math.factorial(d - i) / math.factorial(d + 1)
        thetas.append(c / beta)
    return np.stack(thetas)  # [3, 3] increasing power


THETAS = _calc_thetas(2)


def _pack_idx16(flat):
    """Q7 layout: idx i -> partition i%16, free i//16; replicated x8 groups."""
    n = len(flat)
    m16 = n // 16
    arr = flat.astype(np.int16).reshape(m16, 16).T  # [16, m16]
    return np.tile(arr, (8, 1))  # [128, m16]


def _greedy_balance(cnt4, par, cfg):
    """Assign nodes to BLK-sized blocks minimizing max per-(rel,parity) count.

    cnt4: [NPC, 4] in-degree by (rel, src-parity). par: [NPC] this node's own
    parity class (0/1) — blocks hold exactly BLK/2 of each, and the node's
    position parity within the block equals its class. Returns local padded
    position per node.
    """
    c = cfg
    order = np.argsort(-cnt4.sum(1), kind="stable")
    blk_cnt = np.zeros((c.NBLK, 4), np.int64)
    blk_np = np.zeros((c.NBLK, 2), np.int64)
    pos = np.zeros(c.NPC, np.int64)
    BIG = 1 << 40
    half = c.BLK // 2
    for n in order:
        p = par[n]
        score = (blk_cnt + cnt4[n]).max(axis=1) + blk_cnt.max(axis=1)
        score[blk_np[:, p] >= half] = BIG
        b = int(np.argmin(score))
        pos[n] = b * c.BLK + 2 * blk_np[b, p] + p
        blk_np[b, p] += 1
        blk_cnt[b] += cnt4[n]
    return pos


def _prep_relation(src, dst, cfg, POS):
    """Per-relation host preprocessing: per-core cell-sorted edge slots."""
    c = cfg
    deg = np.bincount(dst, minlength=c.N).astype(np.float32)
    dinv = np.clip(deg, 1.0, None) ** -0.5

    src_pad = POS[src]
    owner = dst // c.NPC
    d_pos = POS[dst]
    all_slots = []
    lh = 0
    for core in range(c.NCORES):
        m = owner == core
        s_p = src_pad[m]
        d_loc = d_pos[m] - core * c.NPAD
        blk = d_loc // c.BLK
        off = (d_loc % c.BLK).astype(np.float32)
        parn = (s_p % 2).astype(np.int64)
        idxv = s_p // 2
        key = blk * 2 + parn
        order = np.lexsort((idxv, key))
        key_s, idx_s, off_s = key[order], idxv[order], off[order]
        counts = np.bincount(key_s, minlength=c.NBLK * 2)
        starts = np.concatenate(([0], np.cumsum(counts)[:-1]))
        within = np.arange(len(key_s)) - starts[key_s]
        lh = max(lh, int(counts.max()) if len(counts) else 0)
        all_slots.append((key_s, within, idx_s, off_s))
    return all_slots, dinv, lh


def _finalize_relation(all_slots, dinv, cfg, LH, POS):
    c = cfg
    NS = LH // 128
    per_core = []
    for core in range(c.NCORES):
        key_s, within, idx_s, off_s = all_slots[core]
        idx_slots = np.zeros((c.NBLK * 2, LH), np.int64)
        off_slots = np.full((c.NBLK * 2, LH), 255.0, np.float32)
        idx_slots[key_s, within] = idx_s
        off_slots[key_s, within] = off_s

        # idx_packed: calls are (parity, 7-block group) = G*LH idxs each,
        # parity-major. Cell (b, par) occupies slots [(b%G)*LH, +LH) of call
        # (par, b//G). Pad slots gather row 0 (their one-hot cols are 0).
        sl = idx_slots.reshape(c.NBLK, 2, LH)
        parts = []
        for par in range(2):
            for g in range(c.NGRP):
                call_flat = sl[g * c.G : (g + 1) * c.G, par, :].reshape(-1)
                parts.append(_pack_idx16(call_flat))
        idx_packed = np.concatenate(parts, axis=1)

        # dstoff: [p, b*2*NS + par*NS + s] = off_slots[b*2+par, s*128+p]
        dstoff = (
            off_slots.reshape(c.NBLK, 2, NS, 128)
            .transpose(3, 0, 1, 2)
            .reshape(128, c.NBLK * 2 * NS)
            .astype(ml_dtypes.bfloat16)
        )

        lpos = POS[core * c.NPC : (core + 1) * c.NPC] - core * c.NPAD
        dv = np.zeros(c.NPAD, np.float32)
        dv[lpos] = dinv[core * c.NPC : (core + 1) * c.NPC]
        dinvT = np.broadcast_to(dv, (c.H, c.NPAD))
        dinv_part = dv.reshape(c.NPAD // 128, 128).T.copy()  # [128, 49]
        per_core.append(
            dict(dinvT=dinvT, dinv_part=dinv_part, idx=idx_packed, dstoff=dstoff)
        )
    return per_core


def _build(cfg, LH):
    """Build the SPMD Bass graph (identical on all cores)."""
    c = cfg
    NS = LH // 128
    H = c.H
    NQ = 4                            # SWDGE queues
    CALL = c.G * LH                   # idxs per gather call
    M16 = CALL // 16                  # idx-tile cols per call

    nc = bacc.Bacc("TRN2", target_bir_lowering=False, debug=False,
                   num_devices=c.NCORES, num_swdge_queues=NQ,
                   dynamic_dma_scratch_size=32768)

    dram_in = {}

    def din(name, shape, dtype=F32):
        dram_in[name] = nc.dram_tensor(name, list(shape), dtype,
                                       kind="ExternalInput")
        return dram_in[name]

    for r in range(2):
        din(f"featT{r}", (c.F, c.NPAD), BF16)
        din(f"dinv_part{r}", (128, c.NPAD // 128))
        din(f"idx{r}", (128, 2 * c.NGRP * M16), I16)
        din(f"dstoff{r}", (128, c.NBLK * 2 * NS), BF16)
        din(f"W1_{r}", (c.F, H), BF16)
        din(f"W2_{r}", (H, H), BF16)
        din(f"b1_{r}", (H, 1))
        din(f"b2_{r}", (H, 1))
    din("dinvT", (128, c.NPAD), BF16)
    din("Vk", (H, 3 * H), BF16)
    din("ident_bf", (128, 128), BF16)
    din("b3x2", (H, 1))
    din("ident", (128, 128))
    din("iotax", (128, c.BLK * 2 * NS), BF16)

    out_t = nc.dram_tensor("out", [c.NPAD, H], F32, kind="ExternalOutput")

    rg = [list(range(c.NCORES))]

    with tile.TileContext(nc) as tc:
        with (
            tc.tile_pool(name="const", bufs=1) as constp,
            tc.tile_pool(name="dram", bufs=1, space="DRAM") as dramp,
            tc.tile_pool(name="feat", bufs=3) as featp,
            tc.tile_pool(name="h1", bufs=2) as h1p,
            tc.tile_pool(name="idxp", bufs=6) as idxp,
            tc.tile_pool(name="oh", bufs=4) as ohp,
            tc.tile_pool(name="sc", bufs=4) as scp,
            tc.tile_pool(name="stg", bufs=3) as stgp,
            tc.tile_pool(name="psmlp", bufs=2, space="PSUM") as psmlp,
            tc.tile_pool(name="psagg", bufs=2, space="PSUM") as psagg,
            tc.tile_pool(name="psvk", bufs=2, space="PSUM") as psvk,
            tc.tile_pool(name="psmisc", bufs=2, space="PSUM") as psmisc,
        ):
            # ---- constants to SBUF ----
            def load_const(name, shape, dtype=F32):
                t = constp.tile(list(shape), dtype, name=f"c_{name}")
                nc.sync.dma_start(out=t[:], in_=dram_in[name].ap()[:])
                return t

            W1 = [load_const(f"W1_{r}", (c.F, H), BF16) for r in range(2)]
            W2 = [load_const(f"W2_{r}", (H, H), BF16) for r in range(2)]
            b1 = [load_const(f"b1_{r}", (H, 1)) for r in range(2)]
            b2 = [load_const(f"b2_{r}", (H, 1)) for r in range(2)]
            dinvT2 = load_const("dinvT", (128, c.NPAD), BF16)
            dinv_part = [load_const(f"dinv_part{r}", (128, c.NPAD // 128))
                         for r in range(2)]
            dstoff = [load_const(f"dstoff{r}", (128, c.NBLK * 2 * NS), BF16)
                      for r in range(2)]
            Vk = load_const("Vk", (H, 3 * H), BF16)
            ident_bf = load_const("ident_bf", (128, 128), BF16)
            b3x2 = load_const("b3x2", (H, 1))
            ident = load_const("ident", (128, 128))
            iotax = load_const("iotax", (128, c.BLK, 2 * NS), BF16)

            # ---- persistent SBUF state ----
            P0T = [constp.tile([H, c.NPAD], BF16, name=f"P0T{r}") for r in range(2)]
            P1T = [constp.tile([H, c.NPAD], BF16, name=f"P1T{r}") for r in range(2)]
            outaccT = constp.tile([H, c.NPAD], BF16, name="outaccT")
            # gathered-row slots: [parity][buf] -> [128, G*NS, H]
            NSLOT = 3
            GTS = [[constp.tile([128, c.G * NS, H], BF16, name=f"gts{p}_{i}")
                    for i in range(NSLOT)] for p in range(2)]

            # ---- internal DRAM ----
            agin = [[dramp.tile([c.NPAD // 128, 128, H], BF16,
                                name=f"agin{r}_{hp}")
                     for hp in range(2)] for r in range(2)]
            table = [[dramp.tile([c.NROW, 2 * H], BF16, name=f"table{r}_{hp}",
                                 addr_space="Shared")
                      for hp in range(2)] for r in range(2)]

            CHUNKS = []
            pos = 0
            while pos < c.NPAD:
                w = min(512, c.NPAD - pos)
                CHUNKS.append((pos, w))
                pos += w

            def write_scaled(PT, r, agin_t):
                """agin rows (node-major) = dinv * PT columns, via PE
                transpose of 128-node pieces + ACT per-partition scale,
                staged 4 pieces per DMA."""
                npc128 = c.NPAD // 128
                stg = None
                for j in range(npc128):
                    tp = psmisc.tile([128, H], BF16, name="tp", tag="misc")
                    nc.tensor.transpose(tp[:], PT[:, j * 128 : (j + 1) * 128],
                                        ident_bf[:H, :H])
                    if j % 4 == 0:
                        stg = stgp.tile([128, 4 * H], BF16, name="stg")
                    nc.scalar.activation(stg[:, (j % 4) * H : (j % 4 + 1) * H],
                                         tp[:], AF.Copy,
                                         scale=dinv_part[r][:, j : j + 1])
                    if j % 4 == 3 or j == npc128 - 1:
                        j0 = j - j % 4
                        nj = j % 4 + 1
                        dst = agin_t[j0 : j0 + nj, :, :].transpose([1, 0, 2])
                        nc.sync.dma_start(out=dst, in_=stg[:, : nj * H])

            def leaky(out_ap, in_ap, bias_ap, w):
                """out = lrelu(in + bias), via DVE (sim lacks ACT Lrelu)."""
                t = h1p.tile([H, 512], F32, name="lk", tag="lk")
                nc.vector.tensor_tensor(out=t[:, :w], in0=in_ap,
                                        in1=bias_ap.to_broadcast((H, w)),
                                        op=mybir.AluOpType.add)
                nc.vector.scalar_tensor_tensor(
                    out=out_ap, in0=t[:, :w], scalar=0.01, in1=t[:, :w],
                    op0=mybir.AluOpType.mult, op1=mybir.AluOpType.max)

            def mlp(r):
                for (p0, w) in CHUNKS:
                    ft = featp.tile([c.F, 512], BF16, name="ft")
                    nc.sync.dma_start(out=ft[:, :w],
                                      in_=dram_in[f"featT{r}"].ap()[:, p0 : p0 + w])
                    ps1 = psmlp.tile([H, 512], F32, name="ps1", tag="mlp")
                    nc.tensor.matmul(ps1[:, :w], W1[r][:], ft[:, :w],
                                     start=True, stop=True)
                    h1t = h1p.tile([H, 512], BF16, name="h1t")
                    leaky(h1t[:, :w], ps1[:, :w], b1[r][:], w)
                    ps2 = psmlp.tile([H, 512], F32, name="ps2", tag="mlp")
                    nc.tensor.matmul(ps2[:, :w], W2[r][:], h1t[:, :w],
                                     start=True, stop=True)
                    leaky(P0T[r][:, p0 : p0 + w], ps2[:, :w], b2[r][:], w)
                write_scaled(P0T[r], r, agin[r][0])

            def allgather(r, hp):
                nc.gpsimd.collective_compute(
                    "AllGather",
                    mybir.AluOpType.bypass,
                    replica_groups=rg,
                    ins=[agin[r][hp][:].opt()],
                    outs=[table[r][hp][:].opt()],
                )

            def prop(r, hop):
                """hop=1: P1T = L~ P0T (+ write scaled1). hop=2: fused output.

                Gather calls cover 7 dst blocks x one src parity (G*LH idxs,
                G*LH/16+1 descs per engine <= 1024-desc SWDGE ring). Calls
                rotate over 4 SWDGE queues; parities of a group pair up so
                aggregation consumes both.
                """
                tab = table[r][hop - 1]
                PTin = P0T[r] if hop == 1 else P1T[r]

                def issue_call(g):
                    gts = []
                    for par in range(2):
                        it = idxp.tile([128, M16], I16, name="it")
                        ci = (par * c.NGRP + g) * M16
                        nc.sync.dma_start(
                            out=it[:],
                            in_=dram_in[f"idx{r}"].ap()[:, ci : ci + M16])
                        gt = GTS[par][g % NSLOT]
                        src_ap = (tab[:, 0:H] if par == 0 else tab[:, H:])
                        _dma_gather_narrow(
                            nc.gpsimd, gt[:, :, :], src_ap, it[:],
                            CALL, CALL, H, 2 * H,
                            queue_num=0)
                        gts.append(gt)
                    return gts

                def do_group(g, gts):
                    bank = psagg.tile([H, c.GW], F32, name="bank", tag="agg")
                    for gg in range(c.G):
                        b = g * c.G + gg
                        col0 = b * 2 * NS
                        # one-hot transposed [128, BLK, 2NS]: both DVE input
                        # streams are inner-contiguous 16-bit (2x perf mode).
                        oh = ohp.tile([128, c.BLK, 2 * NS], BF16, name="oh")
                        nc.vector.tensor_tensor(
                            out=oh[:],
                            in0=dstoff[r][:, col0 : col0 + 2 * NS][:, None, :]
                                .to_broadcast((128, c.BLK, 2 * NS)),
                            in1=iotax[:],
                            op=mybir.AluOpType.is_equal,
                        )
                        os = slice(gg * c.BLK, (gg + 1) * c.BLK)
                        k = 0
                        for par in range(2):
                            for s in range(NS):
                                nc.tensor.matmul(
                                    bank[:, os],
                                    gts[par][:, gg * NS + s, :],
                                    oh[:, :, par * NS + s],
                                    start=(k == 0),
                                    stop=(k == 2 * NS - 1),
                                )
                                k += 1
                    finish_group(g, bank)

                def finish_group(g, bank):
                    cs = slice(g * c.GW, (g + 1) * c.GW)
                    tmp = scp.tile([H, c.GW], BF16, name="tmp")
                    nc.vector.tensor_tensor(out=tmp[:], in0=bank[:],
                                            in1=dinvT2[r * H : (r + 1) * H, cs],
                                            op=mybir.AluOpType.mult)
                    if hop == 1:
                        nc.vector.tensor_tensor(out=P1T[r][:, cs],
                                                in0=PTin[:, cs], in1=tmp[:],
                                                op=mybir.AluOpType.subtract)
                    else:
                        p2 = scp.tile([H, c.GW], BF16, name="p2")
                        nc.vector.tensor_tensor(out=p2[:], in0=PTin[:, cs],
                                                in1=tmp[:],
                                                op=mybir.AluOpType.subtract)
                        op_ps = psvk.tile([H, c.GW], F32, name="opps", tag="vk")
                        nc.tensor.matmul(op_ps[:], Vk[:, 0:H], P0T[r][:, cs],
                                         start=True, stop=False)
                        nc.tensor.matmul(op_ps[:], Vk[:, H : 2 * H],
                                         P1T[r][:, cs], start=False, stop=False)
                        nc.tensor.matmul(op_ps[:], Vk[:, 2 * H : 3 * H], p2[:],
                                         start=False, stop=True)
                        if r == 0:
                            nc.vector.tensor_copy(out=outaccT[:, cs],
                                                  in_=op_ps[:])
                        else:
                            nc.vector.tensor_add(out=outaccT[:, cs],
                                                 in0=outaccT[:, cs],
                                                 in1=op_ps[:])

                LOOKAHEAD = 2
                pend = [issue_call(g) for g in range(min(LOOKAHEAD, c.NGRP))]
                for g in range(c.NGRP):
                    if g + LOOKAHEAD < c.NGRP:
                        pend.append(issue_call(g + LOOKAHEAD))
                    do_group(g, pend[g])
                if hop == 1:
                    write_scaled(P1T[r], r, agin[r][1])

            def final():
                npc128 = c.NPAD // 128
                for (p0, w) in CHUNKS:
                    lr = scp.tile([H, 512], F32, name="lr", tag="lr")
                    leaky(lr[:, :w], outaccT[:, p0 : p0 + w], b3x2[:], w)
                    stg = stgp.tile([128, 4 * H], F32, name="stgo", tag="so")
                    nj = w // 128
                    for k in range(nj):
                        tp = psmisc.tile([128, H], F32, name="tpo", tag="misc")
                        nc.tensor.transpose(tp[:], lr[:, k * 128 : (k + 1) * 128],
                                            ident[:H, :H])
                        nc.vector.tensor_copy(out=stg[:, k * H : (k + 1) * H],
                                              in_=tp[:])
                    dst = (out_t.ap()[p0 : p0 + w, :]
                           .rearrange("(k p) c -> p k c", p=128))
                    nc.sync.dma_start(out=dst, in_=stg[:, : nj * H])

            mlp(0)
            allgather(0, 0)
            mlp(1)
            allgather(1, 0)
            prop(0, 1)
            allgather(0, 1)
            prop(1, 1)
            allgather(1, 1)
            prop(0, 2)
            prop(1, 2)
            final()

    nc.compile()
    return nc


def _prepare(inputs, cfg):
    c = cfg
    W3 = inputs["W3"]
    H = c.H
    V = np.zeros((H, 3 * H), np.float32)
    for k in range(3):
        acc = np.zeros((H, H), np.float64)
        for i in range(3):
            acc += THETAS[i][k] * W3[i * H : (i + 1) * H].astype(np.float64)
        V[:, k * H : (k + 1) * H] = acc.astype(np.float32)

    srcs = [np.asarray(inputs["src_r1"]).astype(np.int64),
            np.asarray(inputs["src_r2"]).astype(np.int64)]
    dsts = [np.asarray(inputs["dst_r1"]).astype(np.int64),
            np.asarray(inputs["dst_r2"]).astype(np.int64)]

    # Node->block balancing permutation (shared by both relations): minimizes
    # the max per-(blk, rel, src-parity) in-degree, i.e. the gather pad waste.
    # Each node's parity class is its global index % 2, a static property the
    # balancer can see (position parity within the block == class).
    POS = np.zeros(c.N, np.int64)
    par_all = np.arange(c.N) % 2
    for core in range(c.NCORES):
        cnt4 = np.zeros((c.NPC, 4), np.int64)
        for r in range(2):
            m = dsts[r] // c.NPC == core
            d_loc = dsts[r][m] - core * c.NPC
            sp = (srcs[r][m] % 2)
            np.add.at(cnt4, (d_loc, 2 * r + sp), 1)
        pos = _greedy_balance(cnt4, par_all[core * c.NPC : (core + 1) * c.NPC], c)
        POS[core * c.NPC : (core + 1) * c.NPC] = core * c.NPAD + pos

    rels = []
    LH = 128
    for r in range(2):
        slots, dinv, lh = _prep_relation(srcs[r], dsts[r], c, POS)
        rels.append((slots, dinv))
        LH = max(LH, lh)
    LH = ((LH + 127) // 128) * 128

    percore_r = []
    for r in range(2):
        slots, dinv = rels[r]
        percore_r.append(_finalize_relation(slots, dinv, c, LH, POS))

    ident = np.eye(128, dtype=np.float32)
    NS = LH // 128
    iotax = np.broadcast_to(
        np.arange(c.BLK, dtype=np.float32)[None, :, None],
        (128, c.BLK, 2 * NS),
    ).reshape(128, c.BLK * 2 * NS).copy()

    in_maps = []
    for core in range(c.NCORES):
        m = {}
        for r in range(2):
            pc = percore_r[r][core]
            m[f"dinv_part{r}"] = pc["dinv_part"]
            m[f"idx{r}"] = pc["idx"]
            m[f"dstoff{r}"] = pc["dstoff"]
            suf = "_r1" if r == 0 else "_r2"
            fk = "feat_r1" if r == 0 else "feat_r2"
            lpos = POS[core * c.NPC : (core + 1) * c.NPC] - core * c.NPAD
            ft = np.zeros((c.F, c.NPAD), ml_dtypes.bfloat16)
            ft[:, lpos] = (np.asarray(inputs[fk])[core * c.NPC : (core + 1) * c.NPC]
                           .T.astype(ml_dtypes.bfloat16))
            m[f"featT{r}"] = ft
            m[f"W1_{r}"] = np.asarray(inputs[f"W1{suf}"], np.float32).astype(
                ml_dtypes.bfloat16)
            m[f"W2_{r}"] = np.asarray(inputs[f"W2{suf}"], np.float32).astype(
                ml_dtypes.bfloat16)
            m[f"b1_{r}"] = np.asarray(inputs[f"b1{suf}"], np.float32).reshape(H, 1)
            m[f"b2_{r}"] = np.asarray(inputs[f"b2{suf}"], np.float32).reshape(H, 1)
        m["dinvT"] = np.concatenate(
            [percore_r[0][core]["dinvT"], percore_r[1][core]["dinvT"]], axis=0
        ).astype(ml_dtypes.bfloat16)
        m["Vk"] = V.astype(ml_dtypes.bfloat16)
        m["b3x2"] = (2.0 * np.asarray(inputs["b3"], np.float32)).reshape(H, 1)
        m["ident"] = ident
        m["ident_bf"] = ident.astype(ml_dtypes.bfloat16)
        m["iotax"] = iotax.astype(ml_dtypes.bfloat16)
        in_maps.append(m)
    return in_maps, LH, POS


def _dma_gather_narrow(gp, out_ap, in_ap, idxs_ap, num_idxs, num_idxs_reg,
                       elem_size, elem_step, queue_num=0):
    """bass.BassGpSimd.dma_gather clone allowing elem_size_bytes % 256 != 0.

    The Q7 kernel (dma_gather.cpp gen_descs, non-transpose HBM path) supports
    any payload length; only the row STRIDE must encode as stride_bytes_256.
    Used to gather 128B bf16 rows from a 256B-strided pair-packed table.
    """
    import concourse.ap_utils as ap_utils
    assert idxs_ap.dtype == I16
    assert in_ap.dtype == out_ap.dtype
    assert in_ap.space == bass.MemorySpace.DRAM
    assert idxs_ap.space == bass.MemorySpace.SBUF
    assert out_ap.space == bass.MemorySpace.SBUF
    assert ap_utils.ap_is_contiguous(out_ap.ap[1:])
    assert ap_utils.ap_is_contiguous(idxs_ap.ap[1:])
    assert in_ap.ap[0][0] == elem_step
    assert in_ap.ap[-1][1] == elem_size
    assert out_ap.ap[-1][1] == elem_size
    assert out_ap.ap[0][1] * out_ap.ap[1][1] * 1 >= num_idxs
    stride_bytes = elem_step * mybir.dt.size(in_ap.dtype)
    assert stride_bytes % 256 == 0 and stride_bytes // 256 < 256
    _in_ap = gp.lower_ap_dma(in_ap, for_custom_bir_dma=True)
    _idxs_ap = gp.lower_ap(idxs_ap)
    _out_ap = gp.lower_ap(out_ap)
    return gp.add_instruction(
        mybir.InstDMAGatherAnt(
            name=gp.bass.get_next_instruction_name(),
            ins=[*_in_ap, _idxs_ap, gp.lower_val_access(gp.to_reg(num_idxs_reg))],
            outs=[_out_ap],
            transpose=False,
            num_idxs=num_idxs,
            elem_size=elem_size,
            stride_bytes_256=stride_bytes // 256,
            gen_mode=0,
            single_packet=False,
            queue_num=queue_num,
            sbuf_tokens_per_rank=0,
            sbuf_free_dim_per_rank=0,
            sbuf_free_dim_pad_per_rank=0,
            sbuf_byte_offset=0,
        )
    )


_CACHE = {}


def _install_profile_shim():
    """Provide antenv.axon_hooks (missing in this image) so trace=True works."""
    try:
        from antenv.axon_hooks import get_axon_ntff_profile_hook  # noqa: F401
        return
    except ImportError:
        pass
    import types

    import antenv
    try:
        from trn_agent_boot.trn_boot import _ntff_profile_via_ctypes
        hook = _ntff_profile_via_ctypes("/opt/axon/libaxon_pjrt.so")
    except Exception:
        hook = None
    mod = types.ModuleType("antenv.axon_hooks")
    mod._hook = hook
    mod.get_axon_ntff_profile_hook = lambda: mod._hook

    def _set(h):
        mod._hook = h

    mod.set_axon_ntff_profile_hook = _set
    sys.modules["antenv.axon_hooks"] = mod
    antenv.axon_hooks = mod


def _run(inputs, trace=False, **kw):
    if trace:
        _install_profile_shim()
    cfg = Cfg(N=int(np.asarray(inputs["feat_r1"]).shape[0]))
    in_maps, LH, POS = _prepare(inputs, cfg)
    key = (cfg.N, LH)
    if key not in _CACHE:
        _CACHE[key] = _build(cfg, LH)
    nc = _CACHE[key]
    res = run_bass_kernel_spmd(nc, in_maps, core_ids=list(range(cfg.NCORES)),
                               trace=trace, **kw)
    outs = []
    for core in range(cfg.NCORES):
        lpos = POS[core * cfg.NPC : (core + 1) * cfg.NPC] - core * cfg.NPAD
        outs.append(np.asarray(res.results[core]["out"])[lpos])
    full = np.concatenate(outs, axis=0)
    return full, res


def kernel(**inputs):
    full, _ = _run(inputs, trace=False)
    return full


# revision 19
# speedup vs baseline: 1.7388x; 1.7388x over previous
"""BWGNN_Hetero Trainium2 kernel (8 NeuronCores, SPMD).

Math restructure of the reference:
  - poly_conv with THETAS (Bernstein, D=2) shares the Krylov basis
    P0 = h, P1 = L~ P0, P2 = L~ P1 across all three thetas, where
    L~ x = x - dinv * segsum((x*dinv)[src], dst).
  - concat(h_theta) @ W3 collapses to sum_k P_k @ V_k with
    V_k = sum_i THETA[i][k] * W3[i*H:(i+1)*H] (host-precomputed).
  - out = leaky(sum_r sum_k P_k^r @ V_k + 2*b3)

Sharding: nodes split 8 ways (6250/core, padded to 6272 = 49*128).
Edges live on their dst-owner core, grouped per (64-node dst block,
src-parity) cell. The scaled-feature table is pair-packed: table row
j = [node 2j | node 2j+1] (256B rows, 25088 rows < int16 range), so
gathers use stride 256B with base offset parity*128B and the
all-gather moves only the compact payload (0.8MB/core per hop).
Gathers are issued as large calls (14 cells = 4480 rows each) to
amortize the ~1us SWDGE fixed cost; aggregation is one-hot matmuls
accumulating 7 blocks into a single PSUM bank, with chunked [64,448]
finish ops.
"""

import math
import os
import sys

import ml_dtypes
import numpy as np

for _p in ("/opt/trn_rl_repo", "/root/.axon_site/_ro/trn_rl_repo"):
    if os.path.isdir(_p) and _p not in sys.path:
        sys.path.insert(0, _p)

import concourse.bacc as bacc
import concourse.bass as bass
import concourse.mybir as mybir
import concourse.tile as tile
from concourse.bass_utils import run_bass_kernel_spmd

F32 = mybir.dt.float32
BF16 = mybir.dt.bfloat16
I16 = mybir.dt.int16
AF = mybir.ActivationFunctionType


class Cfg:
    def __init__(self, N=50000, F=128, H=64, NCORES=8):
        self.N = N
        self.F = F
        self.H = H
        self.NCORES = NCORES
        self.NPC = N // NCORES              # real nodes per core
        self.BLK = 64                       # dst nodes per aggregation block
        self.NBLK = -(-self.NPC // self.BLK)
        self.NPAD = self.NBLK * self.BLK    # padded nodes per core
        assert self.NPAD % 128 == 0
        self.NTAB = self.NPAD * NCORES      # padded global table rows
        self.NROW = self.NTAB // 2          # pair-packed table rows
        assert self.NROW <= 32768, "int16 gather index range"
        self.G = 7                          # dst blocks per finish group
        assert self.NBLK % self.G == 0
        self.NGRP = self.NBLK // self.G     # finish groups per relhop
        self.GW = self.G * self.BLK         # finish-chunk width (448)
        self.CS = 8                         # gather-call size in 128-row slots
        self.GS = 80                        # circular slot-buffer size (5|GS, 8|GS)


THETAS = None


def _calc_thetas(d=2):
    thetas = []
    for i in range(d + 1):
        p1 = np.zeros(i + 1)
        p1[i] = 0.5 ** i
        m = d - i
        p2 = np.array([math.comb(m, j) * (-0.5) ** j for j in range(m + 1)])
        c = np.convolve(p1, p2)
        beta = math.factorial(i) * math.factorial(d - i) / math.factorial(d + 1)
        thetas.append(c / beta)
    return np.stack(thetas)  # [3, 3] increasing power


THETAS = _calc_thetas(2)


def _pack_idx16(flat):
    """Q7 layout: idx i -> partition i%16, free i//16; replicated x8 groups."""
    n = len(flat)
    m16 = n // 16
    arr = flat.astype(np.int16).reshape(m16, 16).T  # [16, m16]
    return np.tile(arr, (8, 1))  # [128, m16]


def _greedy_balance(cnt4, par, cfg):
    """Assign nodes to BLK-sized blocks minimizing max per-(rel,parity) count.

    cnt4: [NPC, 4] in-degree by (rel, src-parity). par: [NPC] this node's own
    parity class (0/1) — blocks hold exactly BLK/2 of each, and the node's
    position parity within the block equals its class. Returns local padded
    position per node.
    """
    c = cfg
    order = np.argsort(-cnt4.sum(1), kind="stable")
    blk_cnt = np.zeros((c.NBLK, 4), np.int64)
    blk_np = np.zeros((c.NBLK, 2), np.int64)
    pos = np.zeros(c.NPC, np.int64)
    BIG = 1 << 40
    half = c.BLK // 2
    for n in order:
        p = par[n]
        score = (blk_cnt + cnt4[n]).max(axis=1) + blk_cnt.max(axis=1)
        score[blk_np[:, p] >= half] = BIG
        b = int(np.argmin(score))
        pos[n] = b * c.BLK + 2 * blk_np[b, p] + p
        blk_np[b, p] += 1
        blk_cnt[b] += cnt4[n]
    return pos


def _prep_relation(src, dst, cfg, POS):
    """Per-relation host preprocessing: per-core cell-sorted edge slots."""
    c = cfg
    deg = np.bincount(dst, minlength=c.N).astype(np.float32)
    dinv = np.clip(deg, 1.0, None) ** -0.5

    src_pad = POS[src]
    owner = dst // c.NPC
    d_pos = POS[dst]
    all_slots = []
    lh = 0
    for core in range(c.NCORES):
        m = owner == core
        s_p = src_pad[m]
        d_loc = d_pos[m] - core * c.NPAD
        blk = d_loc // c.BLK
        off = (d_loc % c.BLK).astype(np.float32)
        parn = (s_p % 2).astype(np.int64)
        idxv = s_p // 2
        key = blk * 2 + parn
        order = np.lexsort((idxv, key))
        key_s, idx_s, off_s = key[order], idxv[order], off[order]
        counts = np.bincount(key_s, minlength=c.NBLK * 2)
        starts = np.concatenate(([0], np.cumsum(counts)[:-1]))
        within = np.arange(len(key_s)) - starts[key_s]
        lh = max(lh, int(counts.max()) if len(counts) else 0)
        all_slots.append((key_s, within, idx_s, off_s))
    return all_slots, dinv, lh


def _finalize_relation(all_slots, dinv, cfg, LH, POS):
    c = cfg
    NS = LH // 128
    per_core = []
    for core in range(c.NCORES):
        key_s, within, idx_s, off_s = all_slots[core]
        idx_slots = np.zeros((c.NBLK * 2, LH), np.int64)
        off_slots = np.full((c.NBLK * 2, LH), 255.0, np.float32)
        idx_slots[key_s, within] = idx_s
        off_slots[key_s, within] = off_s

        # idx_packed: parity-major flat slot space (cell (b, par) at slots
        # [b*NS, (b+1)*NS)); gather calls are CS-slot slices, packed per
        # call. Pad slots gather row 0 (their one-hot cols are 0).
        sl = idx_slots.reshape(c.NBLK, 2, LH)
        parts = []
        for par in range(2):
            flat = sl[:, par, :].reshape(-1)
            for k0 in range(0, len(flat), c.CS * 128):
                parts.append(_pack_idx16(flat[k0 : k0 + c.CS * 128]))
        idx_packed = np.concatenate(parts, axis=1)

        # dstoff: [p, b*2*NS + par*NS + s] = off_slots[b*2+par, s*128+p]
        dstoff = (
            off_slots.reshape(c.NBLK, 2, NS, 128)
            .transpose(3, 0, 1, 2)
            .reshape(128, c.NBLK * 2 * NS)
            .astype(ml_dtypes.bfloat16)
        )

        lpos = POS[core * c.NPC : (core + 1) * c.NPC] - core * c.NPAD
        dv = np.zeros(c.NPAD, np.float32)
        dv[lpos] = dinv[core * c.NPC : (core + 1) * c.NPC]
        dinvT = np.broadcast_to(dv, (c.H, c.NPAD))
        dinv_part = dv.reshape(c.NPAD // 128, 128).T.copy()  # [128, 49]
        per_core.append(
            dict(dinvT=dinvT, dinv_part=dinv_part, idx=idx_packed, dstoff=dstoff)
        )
    return per_core


def _build(cfg, LH):
    """Build the SPMD Bass graph (identical on all cores)."""
    c = cfg
    NS = LH // 128
    H = c.H
    NQ = 4                            # SWDGE queues
    TS = c.NBLK * NS                  # slots per (relhop, parity)
    NCALLS = -(-TS // c.CS)           # gather calls per parity
    LASTS = TS - (NCALLS - 1) * c.CS  # slots in last call
    GS = c.GS
    while GS % NS or GS % c.CS:
        GS += c.CS
    ICOL = TS * 8                     # idx cols per parity (TS*128/16)

    nc = bacc.Bacc("TRN2", target_bir_lowering=False, debug=False,
                   num_devices=c.NCORES, num_swdge_queues=NQ,
                   dynamic_dma_scratch_size=16384)

    dram_in = {}

    def din(name, shape, dtype=F32):
        dram_in[name] = nc.dram_tensor(name, list(shape), dtype,
                                       kind="ExternalInput")
        return dram_in[name]

    for r in range(2):
        din(f"featT{r}", (c.F, c.NPAD), BF16)
        din(f"dinv_part{r}", (128, c.NPAD // 128))
        din(f"idx{r}", (128, 2 * ICOL), I16)
        din(f"dstoff{r}", (128, c.NBLK * 2 * NS), BF16)
        din(f"W1_{r}", (c.F, H), BF16)
        din(f"W2_{r}", (H, H), BF16)
        din(f"b1_{r}", (H, 1))
        din(f"b2_{r}", (H, 1))
    din("dinvT", (128, c.NPAD), BF16)
    din("Vk", (H, 3 * H), BF16)
    din("ident_bf", (128, 128), BF16)
    din("b3x2", (H, 1))
    din("ident", (128, 128))
    din("iotax", (128, c.BLK * 2 * NS), BF16)

    out_t = nc.dram_tensor("out", [c.NPAD, H], F32, kind="ExternalOutput")

    rg = [list(range(c.NCORES))]

    with tile.TileContext(nc) as tc:
        with (
            tc.tile_pool(name="const", bufs=1) as constp,
            tc.tile_pool(name="dram", bufs=1, space="DRAM") as dramp,
            tc.tile_pool(name="feat", bufs=3) as featp,
            tc.tile_pool(name="h1", bufs=2) as h1p,
            tc.tile_pool(name="idxp", bufs=2) as idxp,
            tc.tile_pool(name="oh", bufs=4) as ohp,
            tc.tile_pool(name="sc", bufs=4) as scp,
            tc.tile_pool(name="stg", bufs=3) as stgp,
            tc.tile_pool(name="psmlp", bufs=2, space="PSUM") as psmlp,
            tc.tile_pool(name="psagg", bufs=2, space="PSUM") as psagg,
            tc.tile_pool(name="psvk", bufs=2, space="PSUM") as psvk,
            tc.tile_pool(name="psmisc", bufs=2, space="PSUM") as psmisc,
        ):
            # ---- constants to SBUF ----
            def load_const(name, shape, dtype=F32):
                t = constp.tile(list(shape), dtype, name=f"c_{name}")
                nc.sync.dma_start(out=t[:], in_=dram_in[name].ap()[:])
                return t

            W1 = [load_const(f"W1_{r}", (c.F, H), BF16) for r in range(2)]
            W2 = [load_const(f"W2_{r}", (H, H), BF16) for r in range(2)]
            b1 = [load_const(f"b1_{r}", (H, 1)) for r in range(2)]
            b2 = [load_const(f"b2_{r}", (H, 1)) for r in range(2)]
            dinvT2 = load_const("dinvT", (128, c.NPAD), BF16)
            dinv_part = [load_const(f"dinv_part{r}", (128, c.NPAD // 128))
                         for r in range(2)]
            dstoff = [load_const(f"dstoff{r}", (128, c.NBLK * 2 * NS), BF16)
                      for r in range(2)]
            Vk = load_const("Vk", (H, 3 * H), BF16)
            ident_bf = load_const("ident_bf", (128, 128), BF16)
            b3x2 = load_const("b3x2", (H, 1))
            ident = load_const("ident", (128, 128))
            iotax = load_const("iotax", (128, c.BLK, 2 * NS), BF16)

            # ---- persistent SBUF state ----
            P0T = [constp.tile([H, c.NPAD], BF16, name=f"P0T{r}") for r in range(2)]
            P1T = [constp.tile([H, c.NPAD], BF16, name=f"P1T{r}") for r in range(2)]
            outaccT = constp.tile([H, c.NPAD], BF16, name="outaccT")
            # gathered-row circular slot buffers, one per src parity
            GTS = [constp.tile([128, GS, H], BF16, name=f"gts{p}")
                   for p in range(2)]

            # ---- internal DRAM ----
            agin = [[dramp.tile([c.NPAD // 128, 128, H], BF16,
                                name=f"agin{r}_{hp}")
                     for hp in range(2)] for r in range(2)]
            table = [[dramp.tile([c.NROW, 2 * H], BF16, name=f"table{r}_{hp}",
                                 addr_space="Shared")
                      for hp in range(2)] for r in range(2)]

            CHUNKS = []
            pos = 0
            while pos < c.NPAD:
                w = min(512, c.NPAD - pos)
                CHUNKS.append((pos, w))
                pos += w

            def write_scaled(PT, r, agin_t):
                """agin rows (node-major) = dinv * PT columns, via PE
                transpose of 128-node pieces + ACT per-partition scale,
                staged 4 pieces per DMA."""
                npc128 = c.NPAD // 128
                stg = None
                for j in range(npc128):
                    tp = psmisc.tile([128, H], BF16, name="tp", tag="misc")
                    nc.tensor.transpose(tp[:], PT[:, j * 128 : (j + 1) * 128],
                                        ident_bf[:H, :H])
                    if j % 4 == 0:
                        stg = stgp.tile([128, 4 * H], BF16, name="stg")
                    nc.scalar.activation(stg[:, (j % 4) * H : (j % 4 + 1) * H],
                                         tp[:], AF.Copy,
                                         scale=dinv_part[r][:, j : j + 1])
                    if j % 4 == 3 or j == npc128 - 1:
                        j0 = j - j % 4
                        nj = j % 4 + 1
                        dst = agin_t[j0 : j0 + nj, :, :].transpose([1, 0, 2])
                        nc.sync.dma_start(out=dst, in_=stg[:, : nj * H])

            def leaky(out_ap, in_ap, bias_ap, w):
                """out = lrelu(in + bias), via DVE (sim lacks ACT Lrelu)."""
                t = h1p.tile([H, 512], F32, name="lk", tag="lk")
                nc.vector.tensor_tensor(out=t[:, :w], in0=in_ap,
                                        in1=bias_ap.to_broadcast((H, w)),
                                        op=mybir.AluOpType.add)
                nc.vector.scalar_tensor_tensor(
                    out=out_ap, in0=t[:, :w], scalar=0.01, in1=t[:, :w],
                    op0=mybir.AluOpType.mult, op1=mybir.AluOpType.max)

            def mlp(r):
                for (p0, w) in CHUNKS:
                    ft = featp.tile([c.F, 512], BF16, name="ft")
                    nc.sync.dma_start(out=ft[:, :w],
                                      in_=dram_in[f"featT{r}"].ap()[:, p0 : p0 + w])
                    ps1 = psmlp.tile([H, 512], F32, name="ps1", tag="mlp")
                    nc.tensor.matmul(ps1[:, :w], W1[r][:], ft[:, :w],
                                     start=True, stop=True)
                    h1t = h1p.tile([H, 512], BF16, name="h1t")
                    leaky(h1t[:, :w], ps1[:, :w], b1[r][:], w)
                    ps2 = psmlp.tile([H, 512], F32, name="ps2", tag="mlp")
                    nc.tensor.matmul(ps2[:, :w], W2[r][:], h1t[:, :w],
                                     start=True, stop=True)
                    leaky(P0T[r][:, p0 : p0 + w], ps2[:, :w], b2[r][:], w)
                write_scaled(P0T[r], r, agin[r][0])

            def allgather(r, hp):
                nc.gpsimd.collective_compute(
                    "AllGather",
                    mybir.AluOpType.bypass,
                    replica_groups=rg,
                    ins=[agin[r][hp][:].opt()],
                    outs=[table[r][hp][:].opt()],
                )

            qctr = [0]

            def prop(r, hop):
                """hop=1: P1T = L~ P0T (+ write scaled1). hop=2: fused output.

                Gather calls are CS-slot (1024-idx) slices of the parity-major
                slot space: 64 descs per engine = one max-size SWDGE packet
                (single_packet=True; larger packets hang the SDMA). Calls
                rotate over 4 SWDGE queues into per-parity circular slot
                buffers.
                """
                tab = table[r][hop - 1]
                PTin = P0T[r] if hop == 1 else P1T[r]
                it = idxp.tile([128, 2 * ICOL], I16, name="it")
                nc.sync.dma_start(out=it[:], in_=dram_in[f"idx{r}"].ap()[:])
                issued = [0]

                def issue_call(k):
                    ns = c.CS if k < NCALLS - 1 else LASTS
                    n_i = ns * 128
                    for par in range(2):
                        ci = par * ICOL + k * c.CS * 8
                        s0 = (k * c.CS) % GS
                        src_ap = (tab[:, 0:H] if par == 0 else tab[:, H:])
                        _dma_gather_narrow(
                            nc.gpsimd, GTS[par][:, s0 : s0 + ns, :], src_ap,
                            it[:, ci : ci + ns * 8],
                            n_i, n_i, H, 2 * H,
                            queue_num=qctr[0] % NQ)
                        qctr[0] += 1

                def ensure(upto):
                    while issued[0] < min(upto, NCALLS):
                        issue_call(issued[0])
                        issued[0] += 1

                def do_group(g):
                    bank = psagg.tile([H, c.GW], F32, name="bank", tag="agg")
                    for gg in range(c.G):
                        b = g * c.G + gg
                        col0 = b * 2 * NS
                        # one-hot transposed [128, BLK, 2NS]: both DVE input
                        # streams are inner-contiguous 16-bit (2x perf mode).
                        oh = ohp.tile([128, c.BLK, 2 * NS], BF16, name="oh")
                        nc.vector.tensor_tensor(
                            out=oh[:],
                            in0=dstoff[r][:, col0 : col0 + 2 * NS][:, None, :]
                                .to_broadcast((128, c.BLK, 2 * NS)),
                            in1=iotax[:],
                            op=mybir.AluOpType.is_equal,
                        )
                        os = slice(gg * c.BLK, (gg + 1) * c.BLK)
                        k = 0
                        for par in range(2):
                            for s in range(NS):
                                nc.tensor.matmul(
                                    bank[:, os],
                                    GTS[par][:, (b * NS + s) % GS, :],
                                    oh[:, :, par * NS + s],
                                    start=(k == 0),
                                    stop=(k == 2 * NS - 1),
                                )
                                k += 1
                    finish_group(g, bank)

                def finish_group(g, bank):
                    cs = slice(g * c.GW, (g + 1) * c.GW)
                    tmp = scp.tile([H, c.GW], BF16, name="tmp")
                    nc.vector.tensor_tensor(out=tmp[:], in0=bank[:],
                                            in1=dinvT2[r * H : (r + 1) * H, cs],
                                            op=mybir.AluOpType.mult)
                    if hop == 1:
                        nc.vector.tensor_tensor(out=P1T[r][:, cs],
                                                in0=PTin[:, cs], in1=tmp[:],
                                                op=mybir.AluOpType.subtract)
                    else:
                        p2 = scp.tile([H, c.GW], BF16, name="p2")
                        nc.vector.tensor_tensor(out=p2[:], in0=PTin[:, cs],
                                                in1=tmp[:],
                                                op=mybir.AluOpType.subtract)
                        op_ps = psvk.tile([H, c.GW], F32, name="opps", tag="vk")
                        nc.tensor.matmul(op_ps[:], Vk[:, 0:H], P0T[r][:, cs],
                                         start=True, stop=False)
                        nc.tensor.matmul(op_ps[:], Vk[:, H : 2 * H],
                                         P1T[r][:, cs], start=False, stop=False)
                        nc.tensor.matmul(op_ps[:], Vk[:, 2 * H : 3 * H], p2[:],
                                         start=False, stop=True)
                        if r == 0:
                            nc.vector.tensor_copy(out=outaccT[:, cs],
                                                  in_=op_ps[:])
                        else:
                            nc.vector.tensor_add(out=outaccT[:, cs],
                                                 in0=outaccT[:, cs],
                                                 in1=op_ps[:])

                LOOKAHEAD = 4            # calls beyond the group's needs
                for g in range(c.NGRP):
                    need = -(-(g + 1) * c.G * NS // c.CS)
                    ensure(need + LOOKAHEAD)
                    do_group(g)
                if hop == 1:
                    write_scaled(P1T[r], r, agin[r][1])

            def final():
                npc128 = c.NPAD // 128
                for (p0, w) in CHUNKS:
                    lr = scp.tile([H, 512], F32, name="lr", tag="lr")
                    leaky(lr[:, :w], outaccT[:, p0 : p0 + w], b3x2[:], w)
                    stg = stgp.tile([128, 4 * H], F32, name="stgo", tag="so")
                    nj = w // 128
                    for k in range(nj):
                        tp = psmisc.tile([128, H], F32, name="tpo", tag="misc")
                        nc.tensor.transpose(tp[:], lr[:, k * 128 : (k + 1) * 128],
                                            ident[:H, :H])
                        nc.vector.tensor_copy(out=stg[:, k * H : (k + 1) * H],
                                              in_=tp[:])
                    dst = (out_t.ap()[p0 : p0 + w, :]
                           .rearrange("(k p) c -> p k c", p=128))
                    nc.sync.dma_start(out=dst, in_=stg[:, : nj * H])

            mlp(0)
            allgather(0, 0)
            mlp(1)
            allgather(1, 0)
            prop(0, 1)
            allgather(0, 1)
            prop(1, 1)
            allgather(1, 1)
            prop(0, 2)
            prop(1, 2)
            final()

    nc.compile()
    return nc


def _prepare(inputs, cfg):
    c = cfg
    W3 = inputs["W3"]
    H = c.H
    V = np.zeros((H, 3 * H), np.float32)
    for k in range(3):
        acc = np.zeros((H, H), np.float64)
        for i in range(3):
            acc += THETAS[i][k] * W3[i * H : (i + 1) * H].astype(np.float64)
        V[:, k * H : (k + 1) * H] = acc.astype(np.float32)

    srcs = [np.asarray(inputs["src_r1"]).astype(np.int64),
            np.asarray(inputs["src_r2"]).astype(np.int64)]
    dsts = [np.asarray(inputs["dst_r1"]).astype(np.int64),
            np.asarray(inputs["dst_r2"]).astype(np.int64)]

    # Node->block balancing permutation (shared by both relations): minimizes
    # the max per-(blk, rel, src-parity) in-degree, i.e. the gather pad waste.
    # Each node's parity class is its global index % 2, a static property the
    # balancer can see (position parity within the block == class).
    POS = np.zeros(c.N, np.int64)
    par_all = np.arange(c.N) % 2
    for core in range(c.NCORES):
        cnt4 = np.zeros((c.NPC, 4), np.int64)
        for r in range(2):
            m = dsts[r] // c.NPC == core
            d_loc = dsts[r][m] - core * c.NPC
            sp = (srcs[r][m] % 2)
            np.add.at(cnt4, (d_loc, 2 * r + sp), 1)
        pos = _greedy_balance(cnt4, par_all[core * c.NPC : (core + 1) * c.NPC], c)
        POS[core * c.NPC : (core + 1) * c.NPC] = core * c.NPAD + pos

    rels = []
    LH = 128
    for r in range(2):
        slots, dinv, lh = _prep_relation(srcs[r], dsts[r], c, POS)
        rels.append((slots, dinv))
        LH = max(LH, lh)
    LH = ((LH + 127) // 128) * 128

    percore_r = []
    for r in range(2):
        slots, dinv = rels[r]
        percore_r.append(_finalize_relation(slots, dinv, c, LH, POS))

    ident = np.eye(128, dtype=np.float32)
    NS = LH // 128
    iotax = np.broadcast_to(
        np.arange(c.BLK, dtype=np.float32)[None, :, None],
        (128, c.BLK, 2 * NS),
    ).reshape(128, c.BLK * 2 * NS).copy()

    in_maps = []
    for core in range(c.NCORES):
        m = {}
        for r in range(2):
            pc = percore_r[r][core]
            m[f"dinv_part{r}"] = pc["dinv_part"]
            m[f"idx{r}"] = pc["idx"]
            m[f"dstoff{r}"] = pc["dstoff"]
            suf = "_r1" if r == 0 else "_r2"
            fk = "feat_r1" if r == 0 else "feat_r2"
            lpos = POS[core * c.NPC : (core + 1) * c.NPC] - core * c.NPAD
            ft = np.zeros((c.F, c.NPAD), ml_dtypes.bfloat16)
            ft[:, lpos] = (np.asarray(inputs[fk])[core * c.NPC : (core + 1) * c.NPC]
                           .T.astype(ml_dtypes.bfloat16))
            m[f"featT{r}"] = ft
            m[f"W1_{r}"] = np.asarray(inputs[f"W1{suf}"], np.float32).astype(
                ml_dtypes.bfloat16)
            m[f"W2_{r}"] = np.asarray(inputs[f"W2{suf}"], np.float32).astype(
                ml_dtypes.bfloat16)
            m[f"b1_{r}"] = np.asarray(inputs[f"b1{suf}"], np.float32).reshape(H, 1)
            m[f"b2_{r}"] = np.asarray(inputs[f"b2{suf}"], np.float32).reshape(H, 1)
        m["dinvT"] = np.concatenate(
            [percore_r[0][core]["dinvT"], percore_r[1][core]["dinvT"]], axis=0
        ).astype(ml_dtypes.bfloat16)
        m["Vk"] = V.astype(ml_dtypes.bfloat16)
        m["b3x2"] = (2.0 * np.asarray(inputs["b3"], np.float32)).reshape(H, 1)
        m["ident"] = ident
        m["ident_bf"] = ident.astype(ml_dtypes.bfloat16)
        m["iotax"] = iotax.astype(ml_dtypes.bfloat16)
        in_maps.append(m)
    return in_maps, LH, POS


def _dma_gather_narrow(gp, out_ap, in_ap, idxs_ap, num_idxs, num_idxs_reg,
                       elem_size, elem_step, queue_num=0):
    """bass.BassGpSimd.dma_gather clone allowing elem_size_bytes % 256 != 0.

    The Q7 kernel (dma_gather.cpp gen_descs, non-transpose HBM path) supports
    any payload length; only the row STRIDE must encode as stride_bytes_256.
    Used to gather 128B bf16 rows from a 256B-strided pair-packed table.
    """
    import concourse.ap_utils as ap_utils
    assert idxs_ap.dtype == I16
    assert in_ap.dtype == out_ap.dtype
    assert in_ap.space == bass.MemorySpace.DRAM
    assert idxs_ap.space == bass.MemorySpace.SBUF
    assert out_ap.space == bass.MemorySpace.SBUF
    assert ap_utils.ap_is_contiguous(out_ap.ap[1:])
    assert ap_utils.ap_is_contiguous(idxs_ap.ap[1:])
    assert in_ap.ap[0][0] == elem_step
    assert in_ap.ap[-1][1] == elem_size
    assert out_ap.ap[-1][1] == elem_size
    assert out_ap.ap[0][1] * out_ap.ap[1][1] * 1 >= num_idxs
    stride_bytes = elem_step * mybir.dt.size(in_ap.dtype)
    assert stride_bytes % 256 == 0 and stride_bytes // 256 < 256
    _in_ap = gp.lower_ap_dma(in_ap, for_custom_bir_dma=True)
    _idxs_ap = gp.lower_ap(idxs_ap)
    _out_ap = gp.lower_ap(out_ap)
    return gp.add_instruction(
        mybir.InstDMAGatherAnt(
            name=gp.bass.get_next_instruction_name(),
            ins=[*_in_ap, _idxs_ap, gp.lower_val_access(gp.to_reg(num_idxs_reg))],
            outs=[_out_ap],
            transpose=False,
            num_idxs=num_idxs,
            elem_size=elem_size,
            stride_bytes_256=stride_bytes // 256,
            gen_mode=0,
            single_packet=False,
            queue_num=queue_num,
            sbuf_tokens_per_rank=0,
            sbuf_free_dim_per_rank=0,
            sbuf_free_dim_pad_per_rank=0,
            sbuf_byte_offset=0,
        )
    )


_CACHE = {}


def _install_profile_shim():
    """Provide antenv.axon_hooks (missing in this image) so trace=True works."""
    try:
        from antenv.axon_hooks import get_axon_ntff_profile_hook  # noqa: F401
        return
    except ImportError:
        pass
    import types

    import antenv
    try:
        from trn_agent_boot.trn_boot import _ntff_profile_via_ctypes
        hook = _ntff_profile_via_ctypes("/opt/axon/libaxon_pjrt.so")
    except Exception:
        hook = None
    mod = types.ModuleType("antenv.axon_hooks")
    mod._hook = hook
    mod.get_axon_ntff_profile_hook = lambda: mod._hook

    def _set(h):
        mod._hook = h

    mod.set_axon_ntff_profile_hook = _set
    sys.modules["antenv.axon_hooks"] = mod
    antenv.axon_hooks = mod


def _run(inputs, trace=False, **kw):
    if trace:
        _install_profile_shim()
    cfg = Cfg(N=int(np.asarray(inputs["feat_r1"]).shape[0]))
    in_maps, LH, POS = _prepare(inputs, cfg)
    key = (cfg.N, LH)
    if key not in _CACHE:
        _CACHE[key] = _build(cfg, LH)
    nc = _CACHE[key]
    res = run_bass_kernel_spmd(nc, in_maps, core_ids=list(range(cfg.NCORES)),
                               trace=trace, **kw)
    outs = []
    for core in range(cfg.NCORES):
        lpos = POS[core * cfg.NPC : (core + 1) * cfg.NPC] - core * cfg.NPAD
        outs.append(np.asarray(res.results[core]["out"])[lpos])
    full = np.concatenate(outs, axis=0)
    return full, res


def kernel(**inputs):
    full, _ = _run(inputs, trace=False)
    return full


# revision 20
# speedup vs baseline: 1.7904x; 1.0297x over previous
"""BWGNN_Hetero Trainium2 kernel (8 NeuronCores, SPMD).

Math restructure of the reference:
  - poly_conv with THETAS (Bernstein, D=2) shares the Krylov basis
    P0 = h, P1 = L~ P0, P2 = L~ P1 across all three thetas, where
    L~ x = x - dinv * segsum((x*dinv)[src], dst).
  - concat(h_theta) @ W3 collapses to sum_k P_k @ V_k with
    V_k = sum_i THETA[i][k] * W3[i*H:(i+1)*H] (host-precomputed).
  - out = leaky(sum_r sum_k P_k^r @ V_k + 2*b3)

Sharding: nodes split 8 ways (6250/core, padded to 6272 = 49*128).
Edges live on their dst-owner core, grouped per (64-node dst block,
src-parity) cell. The scaled-feature table is pair-packed: table row
j = [node 2j | node 2j+1] (256B rows, 25088 rows < int16 range), so
gathers use stride 256B with base offset parity*128B and the
all-gather moves only the compact payload (0.8MB/core per hop).
Gathers are issued as large calls (14 cells = 4480 rows each) to
amortize the ~1us SWDGE fixed cost; aggregation is one-hot matmuls
accumulating 7 blocks into a single PSUM bank, with chunked [64,448]
finish ops.
"""

import math
import os
import sys

import ml_dtypes
import numpy as np

for _p in ("/opt/trn_rl_repo", "/root/.axon_site/_ro/trn_rl_repo"):
    if os.path.isdir(_p) and _p not in sys.path:
        sys.path.insert(0, _p)

import concourse.bacc as bacc
import concourse.bass as bass
import concourse.mybir as mybir
import concourse.tile as tile
from concourse.bass_utils import run_bass_kernel_spmd

F32 = mybir.dt.float32
BF16 = mybir.dt.bfloat16
I16 = mybir.dt.int16
AF = mybir.ActivationFunctionType


class Cfg:
    def __init__(self, N=50000, F=128, H=64, NCORES=8):
        self.N = N
        self.F = F
        self.H = H
        self.NCORES = NCORES
        self.NPC = N // NCORES              # real nodes per core
        self.BLK = 64                       # dst nodes per aggregation block
        self.NBLK = -(-self.NPC // self.BLK)
        self.NPAD = self.NBLK * self.BLK    # padded nodes per core
        assert self.NPAD % 128 == 0
        self.NTAB = self.NPAD * NCORES      # padded global table rows
        self.NROW = self.NTAB // 2          # pair-packed table rows
        assert self.NROW <= 32768, "int16 gather index range"
        self.G = 7                          # dst blocks per finish group
        assert self.NBLK % self.G == 0
        self.NGRP = self.NBLK // self.G     # finish groups per relhop
        self.GW = self.G * self.BLK         # finish-chunk width (448)
        self.CS = 8                         # gather-call size in 128-row slots
        self.GS = 80                        # circular slot-buffer size (5|GS, 8|GS)


THETAS = None


def _calc_thetas(d=2):
    thetas = []
    for i in range(d + 1):
        p1 = np.zeros(i + 1)
        p1[i] = 0.5 ** i
        m = d - i
        p2 = np.array([math.comb(m, j) * (-0.5) ** j for j in range(m + 1)])
        c = np.convolve(p1, p2)
        beta = math.factorial(i) * math.factorial(d - i) / math.factorial(d + 1)
        thetas.append(c / beta)
    return np.stack(thetas)  # [3, 3] increasing power


THETAS = _calc_thetas(2)


def _pack_idx16(flat):
    """Q7 layout: idx i -> partition i%16, free i//16; replicated x8 groups."""
    n = len(flat)
    m16 = n // 16
    arr = flat.astype(np.int16).reshape(m16, 16).T  # [16, m16]
    return np.tile(arr, (8, 1))  # [128, m16]


def _greedy_balance(cnt4, par, cfg):
    """Assign nodes to BLK-sized blocks minimizing max per-(rel,parity) count.

    cnt4: [NPC, 4] in-degree by (rel, src-parity). par: [NPC] this node's own
    parity class (0/1) — blocks hold exactly BLK/2 of each, and the node's
    position parity within the block equals its class. Returns local padded
    position per node.
    """
    c = cfg
    order = np.argsort(-cnt4.sum(1), kind="stable")
    blk_cnt = np.zeros((c.NBLK, 4), np.int64)
    blk_np = np.zeros((c.NBLK, 2), np.int64)
    pos = np.zeros(c.NPC, np.int64)
    BIG = 1 << 40
    half = c.BLK // 2
    for n in order:
        p = par[n]
        score = (blk_cnt + cnt4[n]).max(axis=1) + blk_cnt.max(axis=1)
        score[blk_np[:, p] >= half] = BIG
        b = int(np.argmin(score))
        pos[n] = b * c.BLK + 2 * blk_np[b, p] + p
        blk_np[b, p] += 1
        blk_cnt[b] += cnt4[n]
    return pos


def _prep_relation(src, dst, cfg, POS):
    """Per-relation host preprocessing: per-core cell-sorted edge slots."""
    c = cfg
    deg = np.bincount(dst, minlength=c.N).astype(np.float32)
    dinv = np.clip(deg, 1.0, None) ** -0.5

    src_pad = POS[src]
    owner = dst // c.NPC
    d_pos = POS[dst]
    all_slots = []
    lh = 0
    for core in range(c.NCORES):
        m = owner == core
        s_p = src_pad[m]
        d_loc = d_pos[m] - core * c.NPAD
        blk = d_loc // c.BLK
        off = (d_loc % c.BLK).astype(np.float32)
        parn = (s_p % 2).astype(np.int64)
        idxv = s_p // 2
        key = blk * 2 + parn
        order = np.lexsort((idxv, key))
        key_s, idx_s, off_s = key[order], idxv[order], off[order]
        counts = np.bincount(key_s, minlength=c.NBLK * 2)
        starts = np.concatenate(([0], np.cumsum(counts)[:-1]))
        within = np.arange(len(key_s)) - starts[key_s]
        lh = max(lh, int(counts.max()) if len(counts) else 0)
        all_slots.append((key_s, within, idx_s, off_s))
    return all_slots, dinv, lh


def _finalize_relation(all_slots, dinv, cfg, LH, POS):
    c = cfg
    NS = LH // 128
    per_core = []
    for core in range(c.NCORES):
        key_s, within, idx_s, off_s = all_slots[core]
        idx_slots = np.zeros((c.NBLK * 2, LH), np.int64)
        off_slots = np.full((c.NBLK * 2, LH), 255.0, np.float32)
        idx_slots[key_s, within] = idx_s
        off_slots[key_s, within] = off_s

        # idx_packed: parity-major flat slot space (cell (b, par) at slots
        # [b*NS, (b+1)*NS)); gather calls are CS-slot slices, packed per
        # call. Pad slots gather row 0 (their one-hot cols are 0).
        sl = idx_slots.reshape(c.NBLK, 2, LH)
        parts = []
        for par in range(2):
            flat = sl[:, par, :].reshape(-1)
            for k0 in range(0, len(flat), c.CS * 128):
                parts.append(_pack_idx16(flat[k0 : k0 + c.CS * 128]))
        idx_packed = np.concatenate(parts, axis=1)

        # dstoff: [p, b*2*NS + par*NS + s] = off_slots[b*2+par, s*128+p]
        dstoff = (
            off_slots.reshape(c.NBLK, 2, NS, 128)
            .transpose(3, 0, 1, 2)
            .reshape(128, c.NBLK * 2 * NS)
            .astype(ml_dtypes.bfloat16)
        )

        lpos = POS[core * c.NPC : (core + 1) * c.NPC] - core * c.NPAD
        dv = np.zeros(c.NPAD, np.float32)
        dv[lpos] = dinv[core * c.NPC : (core + 1) * c.NPC]
        dinvT = np.broadcast_to(dv, (c.H, c.NPAD))
        dinv_part = dv.reshape(c.NPAD // 128, 128).T.copy()  # [128, 49]
        per_core.append(
            dict(dinvT=dinvT, dinv_part=dinv_part, idx=idx_packed, dstoff=dstoff)
        )
    return per_core


def _build(cfg, LH):
    """Build the SPMD Bass graph (identical on all cores)."""
    c = cfg
    NS = LH // 128
    H = c.H
    NQ = 4                            # SWDGE queues
    TS = c.NBLK * NS                  # slots per (relhop, parity)
    NCALLS = -(-TS // c.CS)           # gather calls per parity
    LASTS = TS - (NCALLS - 1) * c.CS  # slots in last call
    GS = c.GS
    while GS % NS or GS % c.CS:
        GS += c.CS
    ICOL = TS * 8                     # idx cols per parity (TS*128/16)

    nc = bacc.Bacc("TRN2", target_bir_lowering=False, debug=False,
                   num_devices=c.NCORES, num_swdge_queues=NQ,
                   dynamic_dma_scratch_size=16384)

    dram_in = {}

    def din(name, shape, dtype=F32):
        dram_in[name] = nc.dram_tensor(name, list(shape), dtype,
                                       kind="ExternalInput")
        return dram_in[name]

    for r in range(2):
        din(f"featT{r}", (c.F, c.NPAD), BF16)
        din(f"dinv_part{r}", (128, c.NPAD // 128))
        din(f"idx{r}", (128, 2 * ICOL), I16)
        din(f"dstoff{r}", (128, c.NBLK * 2 * NS), BF16)
        din(f"W1_{r}", (c.F, H), BF16)
        din(f"W2_{r}", (H, H), BF16)
        din(f"b1_{r}", (H, 1))
        din(f"b2_{r}", (H, 1))
    din("dinvT", (128, c.NPAD), BF16)
    din("Vk", (H, 3 * H), BF16)
    din("ident_bf", (128, 128), BF16)
    din("b3x2", (H, 1))
    din("ident", (128, 128))
    din("iotax", (128, c.BLK * 2 * NS), BF16)

    out_t = nc.dram_tensor("out", [c.NPAD, H], F32, kind="ExternalOutput")

    rg = [list(range(c.NCORES))]

    with tile.TileContext(nc) as tc:
        with (
            tc.tile_pool(name="const", bufs=1) as constp,
            tc.tile_pool(name="dram", bufs=1, space="DRAM") as dramp,
            tc.tile_pool(name="feat", bufs=3) as featp,
            tc.tile_pool(name="h1", bufs=2) as h1p,
            tc.tile_pool(name="idxp", bufs=2) as idxp,
            tc.tile_pool(name="oh", bufs=4) as ohp,
            tc.tile_pool(name="sc", bufs=4) as scp,
            tc.tile_pool(name="stg", bufs=3) as stgp,
            tc.tile_pool(name="psmlp", bufs=2, space="PSUM") as psmlp,
            tc.tile_pool(name="psagg", bufs=2, space="PSUM") as psagg,
            tc.tile_pool(name="psvk", bufs=2, space="PSUM") as psvk,
            tc.tile_pool(name="psmisc", bufs=2, space="PSUM") as psmisc,
        ):
            # ---- constants to SBUF ----
            def load_const(name, shape, dtype=F32):
                t = constp.tile(list(shape), dtype, name=f"c_{name}")
                nc.sync.dma_start(out=t[:], in_=dram_in[name].ap()[:])
                return t

            W1 = [load_const(f"W1_{r}", (c.F, H), BF16) for r in range(2)]
            W2 = [load_const(f"W2_{r}", (H, H), BF16) for r in range(2)]
            b1 = [load_const(f"b1_{r}", (H, 1)) for r in range(2)]
            b2 = [load_const(f"b2_{r}", (H, 1)) for r in range(2)]
            dinvT2 = load_const("dinvT", (128, c.NPAD), BF16)
            dinv_part = [load_const(f"dinv_part{r}", (128, c.NPAD // 128))
                         for r in range(2)]
            dstoff = [load_const(f"dstoff{r}", (128, c.NBLK * 2 * NS), BF16)
                      for r in range(2)]
            Vk = load_const("Vk", (H, 3 * H), BF16)
            ident_bf = load_const("ident_bf", (128, 128), BF16)
            b3x2 = load_const("b3x2", (H, 1))
            ident = load_const("ident", (128, 128))
            iotax = load_const("iotax", (128, c.BLK, 2 * NS), BF16)

            # ---- persistent SBUF state ----
            P0T = [constp.tile([H, c.NPAD], BF16, name=f"P0T{r}") for r in range(2)]
            P1T = [constp.tile([H, c.NPAD], BF16, name=f"P1T{r}") for r in range(2)]
            outaccT = constp.tile([H, c.NPAD], BF16, name="outaccT")
            # gathered-row circular slot buffers, one per src parity
            GTS = [constp.tile([128, GS, H], BF16, name=f"gts{p}")
                   for p in range(2)]

            # ---- internal DRAM ----
            agin = [[dramp.tile([c.NPAD // 128, 128, H], BF16,
                                name=f"agin{r}_{hp}")
                     for hp in range(2)] for r in range(2)]
            table = [[dramp.tile([c.NROW, 2 * H], BF16, name=f"table{r}_{hp}",
                                 addr_space="Shared")
                      for hp in range(2)] for r in range(2)]

            CHUNKS = []
            pos = 0
            while pos < c.NPAD:
                w = min(512, c.NPAD - pos)
                CHUNKS.append((pos, w))
                pos += w

            def write_scaled(PT, r, agin_t):
                """agin rows (node-major) = dinv * PT columns, via PE
                transpose of 128-node pieces + ACT per-partition scale,
                staged 4 pieces per DMA."""
                npc128 = c.NPAD // 128
                stg = None
                for j in range(npc128):
                    tp = psmisc.tile([128, H], BF16, name="tp", tag="misc")
                    nc.tensor.transpose(tp[:], PT[:, j * 128 : (j + 1) * 128],
                                        ident_bf[:H, :H])
                    if j % 4 == 0:
                        stg = stgp.tile([128, 4 * H], BF16, name="stg")
                    nc.scalar.activation(stg[:, (j % 4) * H : (j % 4 + 1) * H],
                                         tp[:], AF.Copy,
                                         scale=dinv_part[r][:, j : j + 1])
                    if j % 4 == 3 or j == npc128 - 1:
                        j0 = j - j % 4
                        nj = j % 4 + 1
                        dst = agin_t[j0 : j0 + nj, :, :].transpose([1, 0, 2])
                        nc.sync.dma_start(out=dst, in_=stg[:, : nj * H])

            def leaky(out_ap, in_ap, bias_ap, w):
                """out = lrelu(in + bias), via DVE (sim lacks ACT Lrelu)."""
                t = h1p.tile([H, 512], F32, name="lk", tag="lk")
                nc.vector.tensor_tensor(out=t[:, :w], in0=in_ap,
                                        in1=bias_ap.to_broadcast((H, w)),
                                        op=mybir.AluOpType.add)
                nc.vector.scalar_tensor_tensor(
                    out=out_ap, in0=t[:, :w], scalar=0.01, in1=t[:, :w],
                    op0=mybir.AluOpType.mult, op1=mybir.AluOpType.max)

            def mlp(r):
                for (p0, w) in CHUNKS:
                    ft = featp.tile([c.F, 512], BF16, name="ft")
                    nc.sync.dma_start(out=ft[:, :w],
                                      in_=dram_in[f"featT{r}"].ap()[:, p0 : p0 + w])
                    ps1 = psmlp.tile([H, 512], F32, name="ps1", tag="mlp")
                    nc.tensor.matmul(ps1[:, :w], W1[r][:], ft[:, :w],
                                     start=True, stop=True)
                    h1t = h1p.tile([H, 512], BF16, name="h1t")
                    leaky(h1t[:, :w], ps1[:, :w], b1[r][:], w)
                    ps2 = psmlp.tile([H, 512], F32, name="ps2", tag="mlp")
                    nc.tensor.matmul(ps2[:, :w], W2[r][:], h1t[:, :w],
                                     start=True, stop=True)
                    leaky(P0T[r][:, p0 : p0 + w], ps2[:, :w], b2[r][:], w)
                write_scaled(P0T[r], r, agin[r][0])

            def allgather(r, hp):
                nc.gpsimd.collective_compute(
                    "AllGather",
                    mybir.AluOpType.bypass,
                    replica_groups=rg,
                    ins=[agin[r][hp][:].opt()],
                    outs=[table[r][hp][:].opt()],
                )

            qctr = [0]

            def prop(r, hop):
                """hop=1: P1T = L~ P0T (+ write scaled1). hop=2: fused output.

                Gather calls are CS-slot (1024-idx) slices of the parity-major
                slot space: 64 descs per engine = one max-size SWDGE packet
                (single_packet=True; larger packets hang the SDMA). Calls
                rotate over 4 SWDGE queues into per-parity circular slot
                buffers.
                """
                tab = table[r][hop - 1]
                PTin = P0T[r] if hop == 1 else P1T[r]
                it = idxp.tile([128, 2 * ICOL], I16, name="it")
                nc.sync.dma_start(out=it[:], in_=dram_in[f"idx{r}"].ap()[:])
                issued = [0]

                def issue_call(k):
                    ns = c.CS if k < NCALLS - 1 else LASTS
                    n_i = ns * 128
                    for par in range(2):
                        ci = par * ICOL + k * c.CS * 8
                        s0 = (k * c.CS) % GS
                        src_ap = (tab[:, 0:H] if par == 0 else tab[:, H:])
                        _dma_gather_narrow(
                            nc.gpsimd, GTS[par][:, s0 : s0 + ns, :], src_ap,
                            it[:, ci : ci + ns * 8],
                            n_i, n_i, H, 2 * H,
                            queue_num=qctr[0] % NQ)
                        qctr[0] += 1

                def ensure(upto):
                    while issued[0] < min(upto, NCALLS):
                        issue_call(issued[0])
                        issued[0] += 1

                def do_group(g):
                    bank = psagg.tile([H, c.GW], F32, name="bank", tag="agg")
                    for gg in range(c.G):
                        b = g * c.G + gg
                        col0 = b * 2 * NS
                        # one-hot transposed [128, BLK, 2NS]: both DVE input
                        # streams are inner-contiguous 16-bit (2x perf mode).
                        oh = ohp.tile([128, c.BLK, 2 * NS], BF16, name="oh")
                        nc.vector.tensor_tensor(
                            out=oh[:],
                            in0=dstoff[r][:, col0 : col0 + 2 * NS][:, None, :]
                                .to_broadcast((128, c.BLK, 2 * NS)),
                            in1=iotax[:],
                            op=mybir.AluOpType.is_equal,
                        )
                        os = slice(gg * c.BLK, (gg + 1) * c.BLK)
                        k = 0
                        for par in range(2):
                            for s in range(NS):
                                nc.tensor.matmul(
                                    bank[:, os],
                                    GTS[par][:, (b * NS + s) % GS, :],
                                    oh[:, :, par * NS + s],
                                    start=(k == 0),
                                    stop=(k == 2 * NS - 1),
                                )
                                k += 1
                    finish_group(g, bank)

                def finish_group(g, bank):
                    cs = slice(g * c.GW, (g + 1) * c.GW)
                    tmp = scp.tile([H, c.GW], BF16, name="tmp")
                    nc.vector.tensor_tensor(out=tmp[:], in0=bank[:],
                                            in1=dinvT2[r * H : (r + 1) * H, cs],
                                            op=mybir.AluOpType.mult)
                    if hop == 1:
                        nc.vector.tensor_tensor(out=P1T[r][:, cs],
                                                in0=PTin[:, cs], in1=tmp[:],
                                                op=mybir.AluOpType.subtract)
                    else:
                        p2 = scp.tile([H, c.GW], BF16, name="p2")
                        nc.vector.tensor_tensor(out=p2[:], in0=PTin[:, cs],
                                                in1=tmp[:],
                                                op=mybir.AluOpType.subtract)
                        op_ps = psvk.tile([H, c.GW], F32, name="opps", tag="vk")
                        nc.tensor.matmul(op_ps[:], Vk[:, 0:H], P0T[r][:, cs],
                                         start=True, stop=False)
                        nc.tensor.matmul(op_ps[:], Vk[:, H : 2 * H],
                                         P1T[r][:, cs], start=False, stop=False)
                        nc.tensor.matmul(op_ps[:], Vk[:, 2 * H : 3 * H], p2[:],
                                         start=False, stop=True)
                        if r == 0:
                            nc.vector.tensor_copy(out=outaccT[:, cs],
                                                  in_=op_ps[:])
                        else:
                            nc.vector.tensor_add(out=outaccT[:, cs],
                                                 in0=outaccT[:, cs],
                                                 in1=op_ps[:])

                LOOKAHEAD = 4            # calls beyond the group's needs
                for g in range(c.NGRP):
                    need = -(-(g + 1) * c.G * NS // c.CS)
                    ensure(need + LOOKAHEAD)
                    do_group(g)
                if hop == 1:
                    write_scaled(P1T[r], r, agin[r][1])

            def final():
                npc128 = c.NPAD // 128
                for (p0, w) in CHUNKS:
                    lr = scp.tile([H, 512], F32, name="lr", tag="lr")
                    leaky(lr[:, :w], outaccT[:, p0 : p0 + w], b3x2[:], w)
                    stg = stgp.tile([128, 4 * H], F32, name="stgo", tag="so")
                    nj = w // 128
                    for k in range(nj):
                        tp = psmisc.tile([128, H], F32, name="tpo", tag="misc")
                        nc.tensor.transpose(tp[:], lr[:, k * 128 : (k + 1) * 128],
                                            ident[:H, :H])
                        nc.vector.tensor_copy(out=stg[:, k * H : (k + 1) * H],
                                              in_=tp[:])
                    dst = (out_t.ap()[p0 : p0 + w, :]
                           .rearrange("(k p) c -> p k c", p=128))
                    nc.sync.dma_start(out=dst, in_=stg[:, : nj * H])

            mlp(0)
            allgather(0, 0)
            mlp(1)
            allgather(1, 0)
            prop(0, 1)
            allgather(0, 1)
            prop(1, 1)
            allgather(1, 1)
            prop(0, 2)
            prop(1, 2)
            final()

    nc.compile()
    return nc


def _prepare(inputs, cfg):
    c = cfg
    W3 = inputs["W3"]
    H = c.H
    V = np.zeros((H, 3 * H), np.float32)
    for k in range(3):
        acc = np.zeros((H, H), np.float64)
        for i in range(3):
            acc += THETAS[i][k] * W3[i * H : (i + 1) * H].astype(np.float64)
        V[:, k * H : (k + 1) * H] = acc.astype(np.float32)

    srcs = [np.asarray(inputs["src_r1"]).astype(np.int64),
            np.asarray(inputs["src_r2"]).astype(np.int64)]
    dsts = [np.asarray(inputs["dst_r1"]).astype(np.int64),
            np.asarray(inputs["dst_r2"]).astype(np.int64)]

    # Node->block balancing permutation (shared by both relations): minimizes
    # the max per-(blk, rel, src-parity) in-degree, i.e. the gather pad waste.
    # Each node's parity class is its global index % 2, a static property the
    # balancer can see (position parity within the block == class).
    POS = np.zeros(c.N, np.int64)
    par_all = np.arange(c.N) % 2
    for core in range(c.NCORES):
        cnt4 = np.zeros((c.NPC, 4), np.int64)
        for r in range(2):
            m = dsts[r] // c.NPC == core
            d_loc = dsts[r][m] - core * c.NPC
            sp = (srcs[r][m] % 2)
            np.add.at(cnt4, (d_loc, 2 * r + sp), 1)
        pos = _greedy_balance(cnt4, par_all[core * c.NPC : (core + 1) * c.NPC], c)
        POS[core * c.NPC : (core + 1) * c.NPC] = core * c.NPAD + pos

    rels = []
    LH = 128
    for r in range(2):
        slots, dinv, lh = _prep_relation(srcs[r], dsts[r], c, POS)
        rels.append((slots, dinv))
        LH = max(LH, lh)
    LH = ((LH + 127) // 128) * 128

    percore_r = []
    for r in range(2):
        slots, dinv = rels[r]
        percore_r.append(_finalize_relation(slots, dinv, c, LH, POS))

    ident = np.eye(128, dtype=np.float32)
    NS = LH // 128
    iotax = np.broadcast_to(
        np.arange(c.BLK, dtype=np.float32)[None, :, None],
        (128, c.BLK, 2 * NS),
    ).reshape(128, c.BLK * 2 * NS).copy()

    in_maps = []
    for core in range(c.NCORES):
        m = {}
        for r in range(2):
            pc = percore_r[r][core]
            m[f"dinv_part{r}"] = pc["dinv_part"]
            m[f"idx{r}"] = pc["idx"]
            m[f"dstoff{r}"] = pc["dstoff"]
            suf = "_r1" if r == 0 else "_r2"
            fk = "feat_r1" if r == 0 else "feat_r2"
            lpos = POS[core * c.NPC : (core + 1) * c.NPC] - core * c.NPAD
            ft = np.zeros((c.F, c.NPAD), ml_dtypes.bfloat16)
            ft[:, lpos] = (np.asarray(inputs[fk])[core * c.NPC : (core + 1) * c.NPC]
                           .T.astype(ml_dtypes.bfloat16))
            m[f"featT{r}"] = ft
            m[f"W1_{r}"] = np.asarray(inputs[f"W1{suf}"], np.float32).astype(
                ml_dtypes.bfloat16)
            m[f"W2_{r}"] = np.asarray(inputs[f"W2{suf}"], np.float32).astype(
                ml_dtypes.bfloat16)
            m[f"b1_{r}"] = np.asarray(inputs[f"b1{suf}"], np.float32).reshape(H, 1)
            m[f"b2_{r}"] = np.asarray(inputs[f"b2{suf}"], np.float32).reshape(H, 1)
        m["dinvT"] = np.concatenate(
            [percore_r[0][core]["dinvT"], percore_r[1][core]["dinvT"]], axis=0
        ).astype(ml_dtypes.bfloat16)
        m["Vk"] = V.astype(ml_dtypes.bfloat16)
        m["b3x2"] = (2.0 * np.asarray(inputs["b3"], np.float32)).reshape(H, 1)
        m["ident"] = ident
        m["ident_bf"] = ident.astype(ml_dtypes.bfloat16)
        m["iotax"] = iotax.astype(ml_dtypes.bfloat16)
        in_maps.append(m)
    return in_maps, LH, POS


def _dma_gather_narrow(gp, out_ap, in_ap, idxs_ap, num_idxs, num_idxs_reg,
                       elem_size, elem_step, queue_num=0):
    """bass.BassGpSimd.dma_gather clone allowing elem_size_bytes % 256 != 0.

    The Q7 kernel (dma_gather.cpp gen_descs, non-transpose HBM path) supports
    any payload length; only the row STRIDE must encode as stride_bytes_256.
    Used to gather 128B bf16 rows from a 256B-strided pair-packed table.
    """
    import concourse.ap_utils as ap_utils
    assert idxs_ap.dtype == I16
    assert in_ap.dtype == out_ap.dtype
    assert in_ap.space == bass.MemorySpace.DRAM
    assert idxs_ap.space == bass.MemorySpace.SBUF
    assert out_ap.space == bass.MemorySpace.SBUF
    assert ap_utils.ap_is_contiguous(out_ap.ap[1:])
    assert ap_utils.ap_is_contiguous(idxs_ap.ap[1:])
    assert in_ap.ap[0][0] == elem_step
    assert in_ap.ap[-1][1] == elem_size
    assert out_ap.ap[-1][1] == elem_size
    assert out_ap.ap[0][1] * out_ap.ap[1][1] * 1 >= num_idxs
    stride_bytes = elem_step * mybir.dt.size(in_ap.dtype)
    assert stride_bytes % 256 == 0 and stride_bytes // 256 < 256
    _in_ap = gp.lower_ap_dma(in_ap, for_custom_bir_dma=True)
    _idxs_ap = gp.lower_ap(idxs_ap)
    _out_ap = gp.lower_ap(out_ap)
    return gp.add_instruction(
        mybir.InstDMAGatherAnt(
            name=gp.bass.get_next_instruction_name(),
            ins=[*_in_ap, _idxs_ap, gp.lower_val_access(gp.to_reg(num_idxs_reg))],
            outs=[_out_ap],
            transpose=False,
            num_idxs=num_idxs,
            elem_size=elem_size,
            stride_bytes_256=stride_bytes // 256,
            gen_mode=0,
            single_packet=True,
            queue_num=queue_num,
            sbuf_tokens_per_rank=0,
            sbuf_free_dim_per_rank=0,
            sbuf_free_dim_pad_per_rank=0,
            sbuf_byte_offset=0,
        )
    )


_CACHE = {}


def _install_profile_shim():
    """Provide antenv.axon_hooks (missing in this image) so trace=True works."""
    try:
        from antenv.axon_hooks import get_axon_ntff_profile_hook  # noqa: F401
        return
    except ImportError:
        pass
    import types

    import antenv
    try:
        from trn_agent_boot.trn_boot import _ntff_profile_via_ctypes
        hook = _ntff_profile_via_ctypes("/opt/axon/libaxon_pjrt.so")
    except Exception:
        hook = None
    mod = types.ModuleType("antenv.axon_hooks")
    mod._hook = hook
    mod.get_axon_ntff_profile_hook = lambda: mod._hook

    def _set(h):
        mod._hook = h

    mod.set_axon_ntff_profile_hook = _set
    sys.modules["antenv.axon_hooks"] = mod
    antenv.axon_hooks = mod


def _run(inputs, trace=False, **kw):
    if trace:
        _install_profile_shim()
    cfg = Cfg(N=int(np.asarray(inputs["feat_r1"]).shape[0]))
    in_maps, LH, POS = _prepare(inputs, cfg)
    key = (cfg.N, LH)
    if key not in _CACHE:
        _CACHE[key] = _build(cfg, LH)
    nc = _CACHE[key]
    res = run_bass_kernel_spmd(nc, in_maps, core_ids=list(range(cfg.NCORES)),
                               trace=trace, **kw)
    outs = []
    for core in range(cfg.NCORES):
        lpos = POS[core * cfg.NPC : (core + 1) * cfg.NPC] - core * cfg.NPAD
        outs.append(np.asarray(res.results[core]["out"])[lpos])
    full = np.concatenate(outs, axis=0)
    return full, res


def kernel(**inputs):
    full, _ = _run(inputs, trace=False)
    return full


# revision 21
# speedup vs baseline: 2.0730x; 1.1578x over previous
"""BWGNN_Hetero Trainium2 kernel (8 NeuronCores, SPMD).

Math restructure of the reference:
  - poly_conv with THETAS (Bernstein, D=2) shares the Krylov basis
    P0 = h, P1 = L~ P0, P2 = L~ P1 across all three thetas, where
    L~ x = x - dinv * segsum((x*dinv)[src], dst).
  - concat(h_theta) @ W3 collapses to sum_k P_k @ V_k with
    V_k = sum_i THETA[i][k] * W3[i*H:(i+1)*H] (host-precomputed).
  - out = leaky(sum_r sum_k P_k^r @ V_k + 2*b3)

Sharding: nodes split 8 ways (6250/core, padded to 6272 = 49*128 blocks).
Edges live on their dst-owner core, grouped per 128-node dst block and
split by src table half (dma_gather indices are int16; the all-gathered
feature table has 50176 rows, so it is addressed as two 25088-row halves).
Aggregation = gathered-rows (lhsT) x one-hot(dstoff) (rhs) matmuls
accumulated in PSUM, producing feature-major agg^T [64,128] per block.
Between hops, scaled features are all-gathered (1.6MB/core).
"""

import math
import os
import sys

import ml_dtypes
import numpy as np

for _p in ("/opt/trn_rl_repo", "/root/.axon_site/_ro/trn_rl_repo"):
    if os.path.isdir(_p) and _p not in sys.path:
        sys.path.insert(0, _p)

import concourse.bacc as bacc
import concourse.bass as bass
import concourse.mybir as mybir
import concourse.tile as tile
from concourse.bass_utils import run_bass_kernel_spmd

F32 = mybir.dt.float32
BF16 = mybir.dt.bfloat16
I16 = mybir.dt.int16
AF = mybir.ActivationFunctionType


class Cfg:
    def __init__(self, N=50000, F=128, H=64, NCORES=8, BPC=None):
        self.N = N
        self.F = F
        self.H = H
        self.NCORES = NCORES
        self.NPC = N // NCORES              # real nodes per core
        self.BLK = 64                       # dst nodes per aggregation block
        self.NBLK = (self.NPC + self.BLK - 1) // self.BLK
        self.NPAD = self.NBLK * self.BLK    # padded nodes per core
        assert self.NPAD % 128 == 0
        self.NTAB = self.NPAD * NCORES      # padded global table rows
        self.HALF = self.NTAB // 2          # rows per gather-table half
        assert self.HALF <= 32768, "int16 gather index range"
        if BPC is None:
            BPC = next(b for b in (7, 5, 4, 3, 2, 1) if self.NBLK % b == 0)
        self.BPC = BPC                      # blocks per gather call
        assert self.NBLK % BPC == 0
        self.NCALL = self.NBLK // BPC


THETAS = None


def _calc_thetas(d=2):
    thetas = []
    for i in range(d + 1):
        p1 = np.zeros(i + 1)
        p1[i] = 0.5 ** i
        m = d - i
        p2 = np.array([math.comb(m, j) * (-0.5) ** j for j in range(m + 1)])
        c = np.convolve(p1, p2)
        beta = math.factorial(i) * math.factorial(d - i) / math.factorial(d + 1)
        thetas.append(c / beta)
    return np.stack(thetas)  # [3, 3] increasing power


THETAS = _calc_thetas(2)


def _pack_idx16(flat, cfg):
    """Q7 layout: idx i -> partition i%16, free i//16; replicated x8 groups."""
    n = len(flat)
    m16 = n // 16
    arr = flat.astype(np.int16).reshape(m16, 16).T  # [16, m16]
    return np.tile(arr, (8, 1))  # [128, m16]


def _greedy_balance(cnt4, cfg):
    """Assign nodes to BLK-sized blocks minimizing max per-(rel,half) count.

    cnt4: [NPC, 4] in-degree by (rel, half). Returns local padded position
    per node. Shared by both relations (the output layout must match).
    """
    c = cfg
    order = np.argsort(-cnt4.sum(1), kind="stable")
    blk_cnt = np.zeros((c.NBLK, 4), np.int64)
    blk_n = np.zeros(c.NBLK, np.int64)
    pos = np.zeros(c.NPC, np.int64)
    BIG = 1 << 40
    for n in order:
        score = (blk_cnt + cnt4[n]).max(axis=1) + blk_cnt.max(axis=1)
        score[blk_n >= c.BLK] = BIG
        b = int(np.argmin(score))
        pos[n] = b * c.BLK + blk_n[b]
        blk_n[b] += 1
        blk_cnt[b] += cnt4[n]
    return pos


def _prep_relation(feat, src, dst, cfg, POS):
    """Per-relation host preprocessing (vectorized, node-permuted)."""
    c = cfg
    deg = np.bincount(dst, minlength=c.N).astype(np.float32)
    dinv = np.clip(deg, 1.0, None) ** -0.5

    src_pad = POS[src]
    owner = dst // c.NPC
    d_pos = POS[dst]
    all_slots = []
    lh = 0
    for core in range(c.NCORES):
        m = owner == core
        s_p = src_pad[m]
        d_loc = d_pos[m] - core * c.NPAD
        blk = d_loc // c.BLK
        off = (d_loc % c.BLK).astype(np.float32)
        half = (s_p >= c.HALF).astype(np.int64)
        idxv = s_p - half * c.HALF
        key = blk * 2 + half
        order = np.lexsort((idxv, key))
        key_s, idx_s, off_s = key[order], idxv[order], off[order]
        counts = np.bincount(key_s, minlength=c.NBLK * 2)
        starts = np.concatenate(([0], np.cumsum(counts)[:-1]))
        within = np.arange(len(key_s)) - starts[key_s]
        lh = max(lh, int(counts.max()) if len(counts) else 0)
        all_slots.append((key_s, within, idx_s, off_s))
    return all_slots, dinv, lh


def _finalize_relation(all_slots, dinv, feat, cfg, LH, POS):
    c = cfg
    NS = -(-LH // 128)
    LHL = NS * 128                      # sentinel-padded layout length
    per_core = []
    for core in range(c.NCORES):
        key_s, within, idx_s, off_s = all_slots[core]
        idx_slots = np.zeros((c.NBLK * 2, LH), np.int64)
        off_slots = np.full((c.NBLK * 2, LHL), 255.0, np.float32)
        idx_slots[key_s, within] = idx_s
        off_slots[key_s, within] = off_s

        # idx_packed: one call per (blk, half), LH idxs each; pad slots
        # gather row 0 (their one-hot columns are zero).
        sl = idx_slots.reshape(c.NBLK, 2, LH)
        parts = []
        for b in range(c.NBLK):
            for h in range(2):
                parts.append(_pack_idx16(sl[b, h], c))
        idx_packed = np.concatenate(parts, axis=1)

        # dstoff: [p, b*2*NS + h*NS + s] = off_slots[b*2+h, s*128+p]
        dstoff = (
            off_slots.reshape(c.NBLK, 2, NS, 128)
            .transpose(3, 0, 1, 2)
            .reshape(128, c.NBLK * 2 * NS)
            .astype(ml_dtypes.bfloat16)
        )

        lpos = POS[core * c.NPC : (core + 1) * c.NPC] - core * c.NPAD
        dv = np.zeros(c.NPAD, np.float32)
        dv[lpos] = dinv[core * c.NPC : (core + 1) * c.NPC]
        dinvT = np.broadcast_to(dv, (c.H, c.NPAD))
        dinv_col = dv.reshape(c.NBLK, c.BLK).T.copy()  # [BLK, NBLK]

        ft = np.zeros((c.F, c.NPAD), np.float32)
        ft[:, lpos] = feat[core * c.NPC : (core + 1) * c.NPC].T
        per_core.append(
            dict(featT=ft, dinvT=dinvT, dinv_col=dinv_col, idx=idx_packed, dstoff=dstoff)
        )
    return per_core


def _build(cfg, LH):
    """Build the SPMD Bass graph (identical on all cores)."""
    c = cfg
    NS = -(-LH // 128)
    H = c.H
    NQ = 4                            # SWDGE queues

    nc = bacc.Bacc("TRN2", target_bir_lowering=False, debug=False,
                   num_devices=c.NCORES, num_swdge_queues=NQ,
                   dynamic_dma_scratch_size=32768)

    dram_in = {}

    def din(name, shape, dtype=F32):
        dram_in[name] = nc.dram_tensor(name, list(shape), dtype,
                                       kind="ExternalInput")
        return dram_in[name]

    for r in range(2):
        din(f"featT{r}", (c.F, c.NPAD))
        din(f"dinv_col{r}", (c.BLK, c.NBLK))
        din(f"idx{r}", (128, c.NBLK * 2 * (LH // 16)), I16)
        din(f"dstoff{r}", (128, c.NBLK * 2 * NS), BF16)
        din(f"W1_{r}", (c.F, H))
        din(f"W2_{r}", (H, H))
        din(f"b1_{r}", (H, 1))
        din(f"b2_{r}", (H, 1))
    din("dinvT", (128, c.NPAD))
    din("Vk", (H, 3 * H), BF16)
    din("ident_bf", (128, 128), BF16)
    din("b3x2", (H, 1))
    din("ident", (128, 128))
    din("iota", (128, 128), BF16)

    out_t = nc.dram_tensor("out", [c.NPAD, H], F32, kind="ExternalOutput")

    rg = [list(range(c.NCORES))]

    with tile.TileContext(nc) as tc:
        with (
            tc.tile_pool(name="const", bufs=1) as constp,
            tc.tile_pool(name="dram", bufs=1, space="DRAM") as dramp,
            tc.tile_pool(name="feat", bufs=3) as featp,
            tc.tile_pool(name="h1", bufs=2) as h1p,
            tc.tile_pool(name="idxp", bufs=8) as idxp,
            tc.tile_pool(name="oh", bufs=2) as ohp,
            tc.tile_pool(name="sc", bufs=4) as scp,
            tc.tile_pool(name="stg", bufs=3) as stgp,
            tc.tile_pool(name="psmlp", bufs=3, space="PSUM") as psmlp,
            tc.tile_pool(name="psagg", bufs=2, space="PSUM") as psagg,
            tc.tile_pool(name="psmisc", bufs=3, space="PSUM") as psmisc,
        ):
            # ---- constants to SBUF ----
            def load_const(name, shape, dtype=F32):
                t = constp.tile(list(shape), dtype, name=f"c_{name}")
                nc.sync.dma_start(out=t[:], in_=dram_in[name].ap()[:])
                return t

            W1 = [load_const(f"W1_{r}", (c.F, H)) for r in range(2)]
            W2 = [load_const(f"W2_{r}", (H, H)) for r in range(2)]
            b1 = [load_const(f"b1_{r}", (H, 1)) for r in range(2)]
            b2 = [load_const(f"b2_{r}", (H, 1)) for r in range(2)]
            dinvT2 = load_const("dinvT", (128, c.NPAD))
            dinv_col = [load_const(f"dinv_col{r}", (c.BLK, c.NBLK)) for r in range(2)]
            dstoff = [load_const(f"dstoff{r}", (128, c.NBLK * 2 * NS), BF16) for r in range(2)]
            Vk = load_const("Vk", (H, 3 * H), BF16)
            ident_bf = load_const("ident_bf", (128, 128), BF16)
            b3x2 = load_const("b3x2", (H, 1))
            ident = load_const("ident", (128, 128))
            iota = load_const("iota", (128, 128), BF16)

            # ---- persistent SBUF state ----
            P0T = [constp.tile([H, c.NPAD], BF16, name=f"P0T{r}") for r in range(2)]
            NSLOT = 16
            GTS = [constp.tile([128, NS, H], BF16, name=f"gtslot{i}")
                   for i in range(NSLOT)]
            for t in GTS:
                nc.gpsimd.memset(t[:], 0.0)
            P1T = [constp.tile([H, c.NPAD], BF16, name=f"P1T{r}") for r in range(2)]
            outaccT = constp.tile([H, c.NPAD], F32, name="outaccT")

            # ---- internal DRAM ----
            agin = [[dramp.tile([c.NPAD, 2 * H], BF16, name=f"agin{r}_{hp}")
                     for hp in range(2)] for r in range(2)]
            table = [[dramp.tile([c.NTAB, 2 * H], BF16, name=f"table{r}_{hp}")
                      for hp in range(2)] for r in range(2)]

            CHUNKS = []
            pos = 0
            while pos < c.NPAD:
                w = min(512, c.NPAD - pos)
                CHUNKS.append((pos, w))
                pos += w

            def write_scaled(PT, r, agin_t):
                """scaled rows (node-major) = dinv * P rows -> DRAM agin."""
                for b in range(c.NBLK):
                    bs = slice(b * c.BLK, (b + 1) * c.BLK)
                    tp = psmisc.tile([c.BLK, H], BF16, name="tp", tag="misc")
                    nc.tensor.transpose(tp[:], PT[:, bs], ident_bf[:H, :H])
                    stg = stgp.tile([c.BLK, H], BF16, name="stg")
                    nc.scalar.activation(stg[:], tp[:], AF.Copy,
                                         scale=dinv_col[r][:, b : b + 1])
                    nc.sync.dma_start(out=agin_t[bs, 0:H], in_=stg[:])

            def leaky(out_ap, in_ap, bias_ap, w):
                """out = lrelu(in + bias), via DVE (sim lacks ACT Lrelu)."""
                t = h1p.tile([H, 512], F32, name="lk", tag="lk")
                nc.vector.tensor_tensor(out=t[:, :w], in0=in_ap,
                                        in1=bias_ap.to_broadcast((H, w)),
                                        op=mybir.AluOpType.add)
                nc.vector.scalar_tensor_tensor(
                    out=out_ap, in0=t[:, :w], scalar=0.01, in1=t[:, :w],
                    op0=mybir.AluOpType.mult, op1=mybir.AluOpType.max)

            def mlp(r):
                for (p0, w) in CHUNKS:
                    ft = featp.tile([c.F, 512], F32, name="ft")
                    nc.sync.dma_start(out=ft[:, :w],
                                      in_=dram_in[f"featT{r}"].ap()[:, p0 : p0 + w])
                    ps1 = psmlp.tile([H, 512], F32, name="ps1", tag="mlp")
                    nc.tensor.matmul(ps1[:, :w], W1[r][:], ft[:, :w],
                                     start=True, stop=True)
                    h1t = h1p.tile([H, 512], F32, name="h1t")
                    leaky(h1t[:, :w], ps1[:, :w], b1[r][:], w)
                    ps2 = psmlp.tile([H, 512], F32, name="ps2", tag="mlp")
                    nc.tensor.matmul(ps2[:, :w], W2[r][:], h1t[:, :w],
                                     start=True, stop=True)
                    leaky(P0T[r][:, p0 : p0 + w], ps2[:, :w], b2[r][:], w)
                write_scaled(P0T[r], r, agin[r][0])

            def allgather(r, hp):
                nc.gpsimd.collective_compute(
                    "AllGather",
                    mybir.AluOpType.bypass,
                    replica_groups=rg,
                    ins=[agin[r][hp][:].opt()],
                    outs=[table[r][hp][:].opt()],
                )

            def prop(r, hop):
                """hop=1: P1T = L~ P0T (+ write scaled1). hop=2: fused output.

                Gathers stream over a flat subtile space per half in calls of
                CALL_SUB subtiles (<=1024 descriptors: the SWDGE carveout ring
                holds dynamic_dma_scratch_size/16 = 1024 descs; a single
                larger call deadlocks on HW). Calls rotate over 4 SWDGE
                queues.
                """
                tab = table[r][hop - 1]
                PTin = P0T[r] if hop == 1 else P1T[r]
                gts = [[None] * c.NBLK for _ in range(2)]
                M16 = LH // 16

                # split each (blk, half) gather into subtile-aligned pieces
                # small enough that several fit in a SWDGE ring (1024 descs),
                # so desc-gen pipelines with the drain instead of stalling.
                SPLIT = [(s0, min(s0 + 3, NS)) for s0 in range(0, NS, 3)]

                def issue_call(b):
                    for h in range(2):
                        it = idxp.tile([128, M16], I16, name="it")
                        ci = (b * 2 + h) * M16
                        nc.sync.dma_start(
                            out=it[:],
                            in_=dram_in[f"idx{r}"].ap()[:, ci : ci + M16])
                        gt = GTS[(b % (NSLOT // 2)) * 2 + h]
                        src_ap = (tab[0 : c.HALF, 0:H] if h == 0
                                  else tab[c.HALF :, 0:H])
                        for (s0, s1) in SPLIT:
                            n_i = min(LH - s0 * 128, (s1 - s0) * 128)
                            _dma_gather_narrow(
                                nc.gpsimd, gt[:, s0:s1, :], src_ap,
                                it[:, s0 * 8 : s0 * 8 + (n_i + 15) // 16],
                                n_i, n_i, H, 2 * H,
                                queue_num=(2 * b + h + s0) % NQ)
                        gts[h][b] = gt

                def do_block(b):
                    bs = slice(b * c.BLK, (b + 1) * c.BLK)
                    col0 = b * 2 * NS
                    oh = ohp.tile([128, 2 * NS, c.BLK], BF16, name="oh")
                    nc.vector.tensor_tensor(
                        out=oh[:],
                        in0=dstoff[r][:, col0 : col0 + 2 * NS][:, :, None]
                            .to_broadcast((128, 2 * NS, c.BLK)),
                        in1=iota[:][:, None, :c.BLK]
                            .to_broadcast((128, 2 * NS, c.BLK)),
                        op=mybir.AluOpType.is_equal,
                    )
                    agg = psagg.tile([H, c.BLK], F32, name="agg")
                    n_mm = 2 * NS
                    k = 0
                    for h in range(2):
                        for s in range(NS):
                            nc.tensor.matmul(
                                agg[:],
                                gts[h][b][:, s, :],
                                oh[:, h * NS + s, :],
                                start=(k == 0),
                                stop=(k == n_mm - 1),
                            )
                            k += 1
                    finish_block(b, bs, agg)

                def finish_block(b, bs, agg):
                    tmp = scp.tile([H, c.BLK], BF16, name="tmp")
                    nc.vector.tensor_tensor(out=tmp[:], in0=agg[:],
                                            in1=dinvT2[r * H : (r + 1) * H, bs],
                                            op=mybir.AluOpType.mult)
                    if hop == 1:
                        nc.vector.tensor_tensor(out=P1T[r][:, bs],
                                                in0=PTin[:, bs], in1=tmp[:],
                                                op=mybir.AluOpType.subtract)
                    else:
                        p2 = scp.tile([H, c.BLK], BF16, name="p2")
                        nc.vector.tensor_tensor(out=p2[:], in0=PTin[:, bs],
                                                in1=tmp[:],
                                                op=mybir.AluOpType.subtract)
                        op_ps = psmisc.tile([H, c.BLK], F32, name="opps", tag="misc")
                        nc.tensor.matmul(op_ps[:], Vk[:, 0:H], P0T[r][:, bs],
                                         start=True, stop=False)
                        nc.tensor.matmul(op_ps[:], Vk[:, H : 2 * H],
                                         P1T[r][:, bs], start=False, stop=False)
                        nc.tensor.matmul(op_ps[:], Vk[:, 2 * H : 3 * H], p2[:],
                                         start=False, stop=True)
                        if r == 0:
                            nc.vector.tensor_copy(out=outaccT[:, bs],
                                                  in_=op_ps[:])
                        else:
                            nc.vector.tensor_add(out=outaccT[:, bs],
                                                 in0=outaccT[:, bs],
                                                 in1=op_ps[:])

                LOOKAHEAD = 6
                for b in range(min(LOOKAHEAD, c.NBLK)):
                    issue_call(b)
                for b in range(c.NBLK):
                    if b + LOOKAHEAD < c.NBLK:
                        issue_call(b + LOOKAHEAD)
                    do_block(b)
                if hop == 1:
                    write_scaled(P1T[r], r, agin[r][1])

            def final():
                for b in range(c.NBLK):
                    bs = slice(b * c.BLK, (b + 1) * c.BLK)
                    lr = scp.tile([H, c.BLK], F32, name="lr")
                    leaky(lr[:], outaccT[:, bs], b3x2[:], c.BLK)
                    tp = psmisc.tile([c.BLK, H], F32, name="tpo", tag="misc")
                    nc.tensor.transpose(tp[:], lr[:], ident[:H, :H])
                    stg = stgp.tile([c.BLK, H], F32, name="stgo")
                    nc.vector.tensor_copy(out=stg[:], in_=tp[:])
                    nc.sync.dma_start(out=out_t.ap()[bs, :], in_=stg[:])

            mlp(0)
            allgather(0, 0)
            mlp(1)
            allgather(1, 0)
            prop(0, 1)
            allgather(0, 1)
            prop(1, 1)
            allgather(1, 1)
            prop(0, 2)
            prop(1, 2)
            final()

    nc.compile()
    return nc


def _prepare_with_cfg(inputs, cfg):
    r = _prepare(inputs, cfg)
    return r[0], r[1]


def _prepare(inputs, cfg):
    c = cfg
    W3 = inputs["W3"]
    H = c.H
    V = np.zeros((H, 3 * H), np.float32)
    for k in range(3):
        acc = np.zeros((H, H), np.float64)
        for i in range(3):
            acc += THETAS[i][k] * W3[i * H : (i + 1) * H].astype(np.float64)
        V[:, k * H : (k + 1) * H] = acc.astype(np.float32)

    srcs = [np.asarray(inputs["src_r1"]).astype(np.int64),
            np.asarray(inputs["src_r2"]).astype(np.int64)]
    dsts = [np.asarray(inputs["dst_r1"]).astype(np.int64),
            np.asarray(inputs["dst_r2"]).astype(np.int64)]

    # Node->block balancing permutation (shared by both relations): minimizes
    # the max per-(blk, rel, src-half) in-degree, i.e. the gather pad waste.
    POS = np.zeros(c.N, np.int64)
    for core in range(c.NCORES):
        cnt4 = np.zeros((c.NPC, 4), np.int64)
        for r in range(2):
            m = dsts[r] // c.NPC == core
            d_loc = dsts[r][m] - core * c.NPC
            half = (srcs[r][m] // (4 * c.NPC)).clip(0, 1)
            np.add.at(cnt4, (d_loc, 2 * r + half), 1)
        pos = _greedy_balance(cnt4, c)
        POS[core * c.NPC : (core + 1) * c.NPC] = core * c.NPAD + pos

    rels = []
    LH = 16
    for r, (fk, sk, dk) in enumerate(
        [("feat_r1", "src_r1", "dst_r1"), ("feat_r2", "src_r2", "dst_r2")]
    ):
        slots, dinv, lh = _prep_relation(
            np.asarray(inputs[fk]), srcs[r], dsts[r], c, POS)
        rels.append((slots, dinv, np.asarray(inputs[fk], np.float32)))
        LH = max(LH, lh)
    LH = ((LH + 15) // 16) * 16

    percore_r = []
    for r in range(2):
        slots, dinv, feat = rels[r]
        percore_r.append(_finalize_relation(slots, dinv, feat, c, LH, POS))

    ident = np.eye(128, dtype=np.float32)
    iota = np.broadcast_to(np.arange(128, dtype=np.float32), (128, 128)).copy()

    in_maps = []
    for core in range(c.NCORES):
        m = {}
        for r in range(2):
            pc = percore_r[r][core]
            m[f"featT{r}"] = pc["featT"]
            m[f"dinv_col{r}"] = pc["dinv_col"]
            m[f"idx{r}"] = pc["idx"]
            m[f"dstoff{r}"] = pc["dstoff"]
            suf = "_r1" if r == 0 else "_r2"
            m[f"W1_{r}"] = np.asarray(inputs[f"W1{suf}"], np.float32)
            m[f"W2_{r}"] = np.asarray(inputs[f"W2{suf}"], np.float32)
            m[f"b1_{r}"] = np.asarray(inputs[f"b1{suf}"], np.float32).reshape(H, 1)
            m[f"b2_{r}"] = np.asarray(inputs[f"b2{suf}"], np.float32).reshape(H, 1)
        m["dinvT"] = np.concatenate(
            [percore_r[0][core]["dinvT"], percore_r[1][core]["dinvT"]], axis=0
        ).copy()
        m["Vk"] = V.astype(ml_dtypes.bfloat16)
        m["b3x2"] = (2.0 * np.asarray(inputs["b3"], np.float32)).reshape(H, 1)
        m["ident"] = ident
        m["ident_bf"] = ident.astype(ml_dtypes.bfloat16)
        m["iota"] = iota.astype(ml_dtypes.bfloat16)
        in_maps.append(m)
    return in_maps, LH, POS


def _dma_gather_narrow(gp, out_ap, in_ap, idxs_ap, num_idxs, num_idxs_reg,
                       elem_size, elem_step, queue_num=0):
    """bass.BassGpSimd.dma_gather clone allowing elem_size_bytes % 256 != 0.

    The Q7 kernel (dma_gather.cpp gen_descs, non-transpose HBM path) supports
    any payload length; only the row STRIDE must encode as stride_bytes_256.
    Used to gather 128B bf16 rows from a 256B-strided table.
    """
    import concourse.ap_utils as ap_utils
    assert idxs_ap.dtype == I16
    assert in_ap.dtype == out_ap.dtype
    assert in_ap.space == bass.MemorySpace.DRAM
    assert idxs_ap.space == bass.MemorySpace.SBUF
    assert out_ap.space == bass.MemorySpace.SBUF
    assert ap_utils.ap_is_contiguous(out_ap.ap[1:])
    assert ap_utils.ap_is_contiguous(idxs_ap.ap[1:])
    assert in_ap.ap[0][0] == elem_step
    assert in_ap.ap[-1][1] == elem_size
    assert out_ap.ap[-1][1] == elem_size
    assert out_ap.ap[0][1] * out_ap.ap[1][1] * 1 >= num_idxs
    stride_bytes = elem_step * mybir.dt.size(in_ap.dtype)
    assert stride_bytes % 256 == 0 and stride_bytes // 256 < 256
    _in_ap = gp.lower_ap_dma(in_ap, for_custom_bir_dma=True)
    _idxs_ap = gp.lower_ap(idxs_ap)
    _out_ap = gp.lower_ap(out_ap)
    return gp.add_instruction(
        mybir.InstDMAGatherAnt(
            name=gp.bass.get_next_instruction_name(),
            ins=[*_in_ap, _idxs_ap, gp.lower_val_access(gp.to_reg(num_idxs_reg))],
            outs=[_out_ap],
            transpose=False,
            num_idxs=num_idxs,
            elem_size=elem_size,
            stride_bytes_256=stride_bytes // 256,
            gen_mode=0,
            single_packet=True,
            queue_num=queue_num,
            sbuf_tokens_per_rank=0,
            sbuf_free_dim_per_rank=0,
            sbuf_free_dim_pad_per_rank=0,
            sbuf_byte_offset=0,
        )
    )


_CACHE = {}


def _install_profile_shim():
    """Provide antenv.axon_hooks (missing in this image) so trace=True works."""
    try:
        from antenv.axon_hooks import get_axon_ntff_profile_hook  # noqa: F401
        return
    except ImportError:
        pass
    import types

    import antenv
    try:
        from trn_agent_boot.trn_boot import _ntff_profile_via_ctypes
        hook = _ntff_profile_via_ctypes("/opt/axon/libaxon_pjrt.so")
    except Exception:
        hook = None
    mod = types.ModuleType("antenv.axon_hooks")
    mod._hook = hook
    mod.get_axon_ntff_profile_hook = lambda: mod._hook

    def _set(h):
        mod._hook = h

    mod.set_axon_ntff_profile_hook = _set
    sys.modules["antenv.axon_hooks"] = mod
    antenv.axon_hooks = mod


def _run(inputs, trace=False, **kw):
    if trace:
        _install_profile_shim()
    cfg = Cfg(N=int(np.asarray(inputs["feat_r1"]).shape[0]))
    in_maps, LH, POS = _prepare(inputs, cfg)
    key = (cfg.N, LH)
    if key not in _CACHE:
        _CACHE[key] = _build(cfg, LH)
    nc = _CACHE[key]
    res = run_bass_kernel_spmd(nc, in_maps, core_ids=list(range(cfg.NCORES)),
                               trace=trace, **kw)
    outs = []
    for core in range(cfg.NCORES):
        lpos = POS[core * cfg.NPC : (core + 1) * cfg.NPC] - core * cfg.NPAD
        outs.append(np.asarray(res.results[core]["out"])[lpos])
    full = np.concatenate(outs, axis=0)
    return full, res


def kernel(**inputs):
    full, _ = _run(inputs, trace=False)
    return full
